# revision 30
# baseline (speedup 1.0000x reference)
"""Multi-head attention (B=2, L=2048, D=1024, H=16) on 8 TRN2 NeuronCores.

Sharding: batch (2) x head-group (4 heads each) = 8 shards.
Each core computes q/k/v projections for its 4 heads, attention, and a
partial output projection (its 256 rows of Wo); host sums the 4 partials
per batch and adds bo.

Device dataflow (per core):
  inputs (host-prepped, bf16 except memory_bias):
    qT  [1024, 2048]  = query[b].T          (d on partitions for matmul)
    mT  [1024, 2048]  = memory[b].T
    wq  [1024, 256]   = Wq[:, J] * 0.125    (scale folded)
    wk, wv [1024, 256];  bq*0.125, bk, bv [1, 256]
    wo  [256, 1024]   = Wo[J, :]
    mb  [16, 128]     = memory_bias[b]  (f32)
  phase 1: qT_h [j, f], kT_h [j, t] (head-dim on partitions), v [t, j]
           (natural), biases folded in via K=1 ones matmuls, and
           v scaled by exp(memory_bias[t]) so the softmax bias drops out:
           softmax(s + b) @ v == (exp(s) @ (v * e^b)) / (exp(s) @ e^b)
  phase 2: sT = kT.T@qT (two heads row-tiled to overlap in the PE array)
           -> exp on ScalarE (the phase-2 bottleneck, kept saturated via
           a triple-buffered psum pool) -> AV matmul with an extra
           e^b column producing the softmax denominator row for free ->
           normalize into a [128, f] head-pair tile -> row-paired output
           projection partial [f, j].
"""

import numpy as np

import concourse.bass as bass
import concourse.tile as tile
from concourse import bacc, mybir
from concourse import bass_utils
from concourse.bass import ts, ds

F32 = mybir.dt.float32
F32R = mybir.dt.float32r
BF16 = mybir.dt.bfloat16

B, LQ, LM, D, H = 2, 2048, 2048, 1024, 16
DH = 64
HPC = 4            # heads per core
JC = HPC * DH      # 256 projection cols per core
NCORE = 8
P = 128
FB = 512           # f-block width
NFB = LQ // FB     # 4
NTC = LM // P      # 16 t-chunks
KD = D // P        # 8 contraction chunks for projections
G = 2              # t-chunks per exp group (psum tile [128, G*512])
NG = NTC // G      # 8 groups

VW = DH + 1        # v columns per head incl. denominator column
VS = DH + 2        # v column stride per head (4B alignment in bf16)


def build_kernel(mm_dt=BF16, with_biases=False):
    MM = mm_dt
    nc = bacc.Bacc("TRN2", target_bir_lowering=False, debug=False)

    qTd = nc.dram_tensor("qT", [D, LQ], MM, kind="ExternalInput").ap()
    mTd = nc.dram_tensor("mT", [D, LM], MM, kind="ExternalInput").ap()
    wqd = nc.dram_tensor("wq", [D, JC], MM, kind="ExternalInput").ap()
    wkd = nc.dram_tensor("wk", [D, JC], MM, kind="ExternalInput").ap()
    wvd = nc.dram_tensor("wv", [D, JC], MM, kind="ExternalInput").ap()
    if with_biases:
        bqd = nc.dram_tensor("bq", [1, JC], MM, kind="ExternalInput").ap()
        bkd = nc.dram_tensor("bk", [1, JC], MM, kind="ExternalInput").ap()
        bvd = nc.dram_tensor("bv", [1, JC], MM, kind="ExternalInput").ap()
    wod = nc.dram_tensor("wo", [JC, D], MM, kind="ExternalInput").ap()
    ebd = nc.dram_tensor("eb", [P, NTC], F32, kind="ExternalInput").ap()
    outd = nc.dram_tensor("out", [LQ, D], F32, kind="ExternalOutput").ap()

    with tile.TileContext(nc) as tc:
        with (
            tc.tile_pool(name="persist", bufs=1) as persist,
            tc.tile_pool(name="vpool", bufs=1) as vpool,
            tc.tile_pool(name="consts", bufs=1) as consts,
        ):
            # ---- constants ----
            ones_f = consts.tile([1, FB], F32)
            nc.vector.memset(ones_f[:], 1.0)
            ones_row = consts.tile([1, FB], MM)      # rhs for bias matmuls
            nc.vector.tensor_copy(ones_row[:], ones_f[:])
            ones_col = consts.tile([1, P], MM)       # lhsT for v-bias / R bcast
            nc.vector.tensor_copy(ones_col[:], ones_f[:, 0:P])
            eb_sb = consts.tile([P, NTC], F32)  # exp(memory_bias), col=tc
            nc.gpsimd.dma_start(eb_sb[:], ebd[:])

            # ---- persistent activations ----
            # qT/kT: per head-pair tile [128 (2 heads x 64 dh), L]
            qTp = [persist.tile([P, LQ], MM, name=f"qTp{i}") for i in range(2)]
            kTp = [persist.tile([P, LM], MM, name=f"kTp{i}") for i in range(2)]
            # v: per t-chunk [128 t, 4 heads x (64 v cols + e^b col + pad)]
            v_sb = [vpool.tile([P, HPC * VS], MM, name=f"v{t}")
                    for t in range(NTC)]
            wop = [persist.tile([P, D], MM, name=f"wop{i}")
                   for i in range(2)]

            # ======= fused projections + attention, software-pipelined =======
            # Emission order (per-engine streams are in-order, so emission
            # order is the schedule):
            #   rounds 0..3:  k/v projections for t-window r, q projection
            #                 for f-block 0 (round 0 only), then fb0's logits
            #                 groups {2r, 2r+1} for both head pairs
            #   fb blocks 1..3: q projection, then 16 logits+exp steps, with
            #                 AV matmuls of previously-completed (fb, hp)
            #                 blocks drained from a FIFO at 4 per step
            #   tail:         remaining AV units + last out-projection
            # PSUM: psl 3x[128, G*FB] (logits pairs + proj/rb/out-proj
            # rotations) + ppv 1 (v, phase 1 only) + psx 2 (AV accumulators).
            wq_sb = persist.tile([P, KD * JC], MM, name="wq")
            bq_sb = persist.tile([1, JC], MM, name="bq")
            wk_sb = persist.tile([P, KD * JC], MM, name="wk")
            wv_sb = persist.tile([P, KD * JC], MM, name="wv")
            bk_sb = persist.tile([1, JC], MM, name="bk")
            bv_sb = persist.tile([1, JC], MM, name="bv")

            with (
                tc.tile_pool(name="mrhs", bufs=2) as mrhsp,
                tc.tile_pool(name="expp", bufs=34) as expp,
                tc.tile_pool(name="attnp", bufs=6) as attnp,
                tc.tile_pool(name="rp", bufs=6) as rp,
                tc.tile_pool(name="rbp", bufs=4) as rbp,
                tc.tile_pool(name="osb", bufs=4) as osb,
                tc.tile_pool(name="psl", bufs=3, space="PSUM") as psl,
            ):
                for w_sb, wd in ((wk_sb, wkd), (wv_sb, wvd)):
                    h_ = KD // 2
                    nc.sync.dma_start(
                        w_sb[:, 0:h_ * JC].rearrange("p (k j) -> p k j", k=h_),
                        wd[ds(0, h_ * P), :].rearrange("(k p) j -> p k j", p=P))
                    nc.gpsimd.dma_start(
                        w_sb[:, h_ * JC:].rearrange("p (k j) -> p k j", k=h_),
                        wd[ds(h_ * P, h_ * P), :].rearrange(
                            "(k p) j -> p k j", p=P))
                if with_biases:
                    nc.gpsimd.dma_start(bk_sb[:], bkd[:])
                    nc.gpsimd.dma_start(bv_sb[:], bvd[:])
                nc.gpsimd.dma_start(
                    wq_sb[:].rearrange("p (k j) -> p k j", k=KD),
                    wqd.rearrange("(k p) j -> p k j", p=P))
                if with_biases:
                    nc.gpsimd.dma_start(bq_sb[:], bqd[:])
                for i in range(2):
                    nc.gpsimd.dma_start(wop[i][:], wod[ds(i * P, P), :])

                # HAM warm-up: keep the PE busy during the initial input
                # DMAs so phase 1 starts at full clock (the activity monitor
                # needs ~3.4us of sustained work to unthrottle)
                junk = persist.tile([P, P], MM, name="junk")
                nc.vector.memset(junk[:].bitcast(F32)[:, 0:P // 2], 1.0)
                wps = psl.tile([P, G * FB], F32, name="warm", tag="pls")
                for i in range(30):
                    nc.tensor.matmul(wps[:, 0:P], junk[:], junk[:],
                                     start=True, stop=True)

                exps = {}      # (fb, hp, h2, g) -> [128, G*FB] bf16 tile
                apairs = {}    # (fb, hp) -> [128, FB] attn pair tile
                work_q = []    # FIFO: ["av", fb, hp, h2, tc_next, av_ap]
                               #       ["op", fb, fc]
                avail = {}     # (fb, hp) -> highest t-chunk with exp emitted

                def kq_proj(w_sb, b_sb, chunks, dstp, col):
                    for hp in range(2):
                        ps = psl.tile([P, G * FB], F32, name="pp", tag="pls")
                        for k in range(KD):
                            nc.tensor.matmul(
                                ps[:, 0:FB],
                                w_sb[:, ds(k * JC + hp * P, P)], chunks[k],
                                start=(k == 0),
                                stop=(not with_biases and k == KD - 1))
                        if with_biases:
                            nc.tensor.matmul(
                                ps[:, 0:FB], b_sb[:, ds(hp * P, P)],
                                ones_row[:], start=False, stop=True)
                        nc.vector.tensor_copy(dstp[hp][:, col], ps[:, 0:FB])

                def q_proj(fb):
                    qt = mrhsp.tile([P, KD * FB], MM, name="qchunk")
                    for k in range(KD):
                        eng = nc.sync if k % 2 == 0 else nc.gpsimd
                        eng.dma_start(qt[:, ts(k, FB)],
                                      qTd[ds(k * P, P), ts(fb, FB)])
                    kq_proj(wq_sb, bq_sb, [qt[:, ts(k, FB)] for k in range(KD)],
                            qTp, ts(fb, FB))

                def finish_unit(u):
                    _, fb, hp, h2, _, av = u
                    dn = rp.tile([1, FB], F32, name="dn")
                    nc.vector.tensor_copy(dn[:], av[ds(DH, 1), :])
                    rf = rp.tile([1, FB], F32, name="rf")
                    nc.vector.reciprocal_approx_fast(rf[:], dn[:])
                    rrow = rp.tile([1, FB], MM, name="rrow")
                    nc.vector.tensor_copy(rrow[:], rf[:])
                    rb_ps = psl.tile([P, G * FB], F32, name="rbps", tag="pls")
                    nc.tensor.matmul(rb_ps[:, 0:FB], ones_col[:], rrow[:],
                                     start=True, stop=True)
                    rb = rbp.tile([DH, FB], F32, name="rb")
                    nc.vector.tensor_copy(rb[:], rb_ps[0:DH, 0:FB])
                    if (fb, hp) not in apairs:
                        apairs[(fb, hp)] = attnp.tile([P, FB], MM,
                                                      name="apair")
                    nc.vector.tensor_tensor(
                        apairs[(fb, hp)][ds(h2 * DH, DH), :],
                        av[0:DH, :], rb[:], op=mybir.AluOpType.mult)
                    if hp == 1 and h2 == 1:
                        for fc in range(4):
                            work_q.append(["op", fb, fc])

                def out_proj_piece(fb, fc):
                    attn = [apairs[(fb, 0)], apairs[(fb, 1)]]
                    o = osb.tile([P, D], F32, name="osb")
                    for jb in range(2):
                        ops = psl.tile([P, G * FB], F32, name="ops",
                                       tag="pls")
                        for hp in range(2):
                            nc.tensor.matmul(
                                ops[:, 0:FB],
                                attn[hp][:, ds(fc * P, P)],
                                wop[hp][:, ts(jb, FB)],
                                start=(hp == 0), stop=(hp == 1))
                        nc.vector.tensor_copy(o[:, ts(jb, FB)],
                                              ops[:, 0:FB])
                    eng = nc.gpsimd if fc % 2 == 0 else nc.sync
                    eng.dma_start(outd[ds(fb * FB + fc * P, P), :], o[:])
                    if fc == 3:
                        apairs.pop((fb, 0))
                        apairs.pop((fb, 1))

                def drain_av(budget):
                    while budget > 0 and work_q:
                        u = work_q[0]
                        if u[0] == "op":
                            out_proj_piece(u[1], u[2])
                            work_q.pop(0)
                            budget -= 3
                            continue
                        _, fb, hp, h2, tcn, av = u
                        if tcn >= avail.get((fb, hp), 0):
                            break   # strict FIFO; head not yet runnable
                        h = 2 * hp + h2
                        nc.tensor.matmul(
                            av[:], v_sb[tcn][:, ds(h * VS, VW)],
                            exps[(fb, hp, h2, tcn // G)][:, ts(tcn % G, FB)],
                            start=(tcn == 0), stop=(tcn == NTC - 1))
                        u[4] += 1
                        budget -= 1
                        if u[4] == NTC:
                            work_q.pop(0)
                            finish_unit(u)

                def logits_step(fb, hp, g, av_budget=4):
                    pls = [psl.tile([P, G * FB], F32, name="pls")
                           for _ in range(2)]
                    for s in range(G):
                        t = g * G + s
                        for h2 in range(2):
                            nc.tensor.matmul(
                                pls[h2][:, ts(s, FB)],
                                kTp[hp][ds(h2 * DH, DH), ts(t, P)],
                                qTp[hp][ds(h2 * DH, DH), ts(fb, FB)],
                                start=True, stop=True)
                    for h2 in range(2):
                        e = expp.tile([P, G * FB], MM, name="exps")
                        nc.scalar.activation(e[:], pls[h2][:],
                                             mybir.ActivationFunctionType.Exp)
                        exps[(fb, hp, h2, g)] = e
                    avail[(fb, hp)] = (g + 1) * G
                    drain_av(av_budget)

                def enqueue_block(fb, hp):
                    for h2 in range(2):
                        av = psx.tile([P, FB], F32, name="av")[0:VW, :]
                        work_q.append(["av", fb, hp, h2, 0, av])

                # ---- phase 1: k/v rounds + fb0 logits ----
                with tc.tile_pool(name="ppv", bufs=1, space="PSUM") as ppv:
                    for rnd in range(NFB):
                        mt = mrhsp.tile([P, KD * FB], MM, name="mchunk")
                        for k in range(KD):
                            eng = nc.sync if k % 2 == 0 else nc.gpsimd
                            eng.dma_start(mt[:, ts(k, FB)],
                                          mTd[ds(k * P, P), ts(rnd, FB)])
                        chunks = [mt[:, ts(k, FB)] for k in range(KD)]
                        kq_proj(wk_sb, bk_sb, chunks, kTp, ts(rnd, FB))
                        if rnd == 0:
                            q_proj(0)
                        for g in (2 * rnd, 2 * rnd + 1):
                            for hp in range(2):
                                logits_step(0, hp, g)
                        for s in range(4):
                            t = rnd * 4 + s
                            psv = ppv.tile([P, JC], F32)
                            for k in range(KD):
                                nc.tensor.matmul(
                                    psv[:], chunks[k][:, ds(s * P, P)],
                                    wv_sb[:, ts(k, JC)],
                                    start=(k == 0),
                                    stop=(not with_biases and k == KD - 1))
                            if with_biases:
                                nc.tensor.matmul(
                                    psv[:], ones_col[:], bv_sb[:],
                                    start=False, stop=True)
                            dst = v_sb[t].rearrange("p (h c) -> p h c", h=HPC)
                            nc.vector.tensor_scalar_mul(
                                dst[:, :, 0:DH],
                                psv[:].rearrange("p (h c) -> p h c", h=HPC),
                                eb_sb[:, ds(t, 1)])
                            for h in range(HPC):
                                nc.vector.tensor_copy(
                                    dst[:, ds(h, 1), ds(DH, 1)],
                                    eb_sb[:, ds(t, 1)])

                # ---- steady state: fb blocks with AV drained in-stream ----
                with tc.tile_pool(name="psx", bufs=2, space="PSUM") as psx:
                    for hp in range(2):
                        enqueue_block(0, hp)
                    q_proj(1)
                    for fb in range(1, NFB):
                        for hp in range(2):
                            enqueue_block(fb, hp)
                            for g in range(NG):
                                logits_step(fb, hp, g,
                                            av_budget=5 if fb < 3 else 8)
                            if hp == 0 and fb < NFB - 1:
                                q_proj(fb + 1)
                    drain_av(10 ** 9)

    nc.compile()
    return nc


_CACHE = {}


def _get_module(with_biases=False):
    key = ("nc", with_biases)
    if key not in _CACHE:
        _CACHE[key] = build_kernel(with_biases=with_biases)
    return _CACHE[key]


def make_in_maps(query, memory, memory_bias, Wq, bq, Wk, bk, Wv, bv, Wo, bo,
                 mm_np=None, with_biases=False):
    if mm_np is None:
        import ml_dtypes
        mm_np = ml_dtypes.bfloat16
    query = np.asarray(query, np.float32)
    memory = np.asarray(memory, np.float32)
    memory_bias = np.asarray(memory_bias, np.float32)
    Wq = np.asarray(Wq, np.float32)
    bq = np.asarray(bq, np.float32)
    Wk = np.asarray(Wk, np.float32)
    bk = np.asarray(bk, np.float32)
    Wv = np.asarray(Wv, np.float32)
    bv = np.asarray(bv, np.float32)
    Wo = np.asarray(Wo, np.float32)
    s = np.float32(DH ** -0.5)

    qT = [np.ascontiguousarray(query[b].T).astype(mm_np) for b in range(B)]
    mT = [np.ascontiguousarray(memory[b].T).astype(mm_np) for b in range(B)]
    in_maps = []
    for c in range(NCORE):
        b, g = divmod(c, 4)
        J = slice(g * JC, (g + 1) * JC)
        m = {
            "qT": qT[b],
            "mT": mT[b],
            "wq": (np.ascontiguousarray(Wq[:, J]) * s).astype(mm_np),
            "wk": np.ascontiguousarray(Wk[:, J]).astype(mm_np),
            "wv": np.ascontiguousarray(Wv[:, J]).astype(mm_np),
            "wo": np.ascontiguousarray(Wo[J, :]).astype(mm_np),
            "eb": np.ascontiguousarray(
                np.exp(memory_bias[b].astype(np.float64)).reshape(
                    NTC, P).T).astype(np.float32),
        }
        if with_biases:
            m["bq"] = (bq[J] * s).reshape(1, JC).astype(mm_np)
            m["bk"] = bk[J].reshape(1, JC).astype(mm_np)
            m["bv"] = bv[J].reshape(1, JC).astype(mm_np)
        in_maps.append(m)
    return in_maps


def gather_output(results, bo):
    bo = np.asarray(bo, np.float32)
    out = np.empty((B, LQ, D), np.float32)
    for b in range(B):
        acc = results[4 * b]["out"].astype(np.float32)
        for g in range(1, 4):
            acc = acc + results[4 * b + g]["out"]
        out[b] = acc + bo
    return out


def kernel(**inputs):
    wb = any(np.any(np.asarray(inputs[b])) for b in ("bq", "bk", "bv"))
    nc = _get_module(with_biases=wb)
    in_maps = make_in_maps(**inputs, with_biases=wb)
    res = bass_utils.run_bass_kernel_spmd(nc, in_maps,
                                          core_ids=list(range(NCORE)))
    return gather_output(res.results, inputs["bo"])


# revision 31
# speedup vs baseline: 1.1578x; 1.1578x over previous
"""Multi-head attention (B=2, L=2048, D=1024, H=16) on 8 TRN2 NeuronCores.

Sharding: batch (2) x head-group (4 heads each) = 8 shards.
Each core computes q/k/v projections for its 4 heads, attention, and a
partial output projection (its 256 rows of Wo); host sums the 4 partials
per batch and adds bo.

Device dataflow (per core):
  inputs (host-prepped, bf16 except memory_bias):
    qT  [1024, 2048]  = query[b].T          (d on partitions for matmul)
    mT  [1024, 2048]  = memory[b].T
    wq  [1024, 256]   = Wq[:, J] * 0.125    (scale folded)
    wk, wv [1024, 256];  bq*0.125, bk, bv [1, 256]
    wo  [256, 1024]   = Wo[J, :]
    mb  [16, 128]     = memory_bias[b]  (f32)
  phase 1: qT_h [j, f], kT_h [j, t] (head-dim on partitions), v [t, j]
           (natural), biases folded in via K=1 ones matmuls, and
           v scaled by exp(memory_bias[t]) so the softmax bias drops out:
           softmax(s + b) @ v == (exp(s) @ (v * e^b)) / (exp(s) @ e^b)
  phase 2: sT = kT.T@qT (two heads row-tiled to overlap in the PE array)
           -> exp on ScalarE (the phase-2 bottleneck, kept saturated via
           a triple-buffered psum pool) -> AV matmul with an extra
           e^b column producing the softmax denominator row for free ->
           normalize into a [128, f] head-pair tile -> row-paired output
           projection partial [f, j].
"""

import numpy as np

import concourse.bass as bass
import concourse.tile as tile
from concourse import bacc, mybir
from concourse import bass_utils
from concourse.bass import ts, ds

F32 = mybir.dt.float32
F32R = mybir.dt.float32r
BF16 = mybir.dt.bfloat16

B, LQ, LM, D, H = 2, 2048, 2048, 1024, 16
DH = 64
HPC = 4            # heads per core
JC = HPC * DH      # 256 projection cols per core
NCORE = 8
P = 128
FB = 512           # f-block width
NFB = LQ // FB     # 4
NTC = LM // P      # 16 t-chunks
KD = D // P        # 8 contraction chunks for projections
G = 2              # t-chunks per exp group (psum tile [128, G*512])
NG = NTC // G      # 8 groups

VW = DH + 1        # v columns per head incl. denominator column
VS = DH + 2        # v column stride per head (4B alignment in bf16)


def build_kernel(mm_dt=BF16, with_biases=False):
    MM = mm_dt
    nc = bacc.Bacc("TRN2", target_bir_lowering=False, debug=False)

    qTd = nc.dram_tensor("qT", [D, LQ], MM, kind="ExternalInput").ap()
    mTd = nc.dram_tensor("mT", [D, LM], MM, kind="ExternalInput").ap()
    wqd = nc.dram_tensor("wq", [D, JC], MM, kind="ExternalInput").ap()
    wkd = nc.dram_tensor("wk", [D, JC], MM, kind="ExternalInput").ap()
    wvd = nc.dram_tensor("wv", [D, JC], MM, kind="ExternalInput").ap()
    if with_biases:
        bqd = nc.dram_tensor("bq", [1, JC], MM, kind="ExternalInput").ap()
        bkd = nc.dram_tensor("bk", [1, JC], MM, kind="ExternalInput").ap()
        bvd = nc.dram_tensor("bv", [1, JC], MM, kind="ExternalInput").ap()
    wod = nc.dram_tensor("wo", [JC, D], MM, kind="ExternalInput").ap()
    ebd = nc.dram_tensor("eb", [P, NTC], F32, kind="ExternalInput").ap()
    outd = nc.dram_tensor("out", [LQ, D], F32, kind="ExternalOutput").ap()

    with tile.TileContext(nc) as tc:
        with (
            tc.tile_pool(name="persist", bufs=1) as persist,
            tc.tile_pool(name="vpool", bufs=1) as vpool,
            tc.tile_pool(name="consts", bufs=1) as consts,
        ):
            # ---- constants ----
            ones_f = consts.tile([1, FB], F32)
            nc.vector.memset(ones_f[:], 1.0)
            ones_row = consts.tile([1, FB], MM)      # rhs for bias matmuls
            nc.vector.tensor_copy(ones_row[:], ones_f[:])
            ones_col = consts.tile([1, P], MM)       # lhsT for v-bias / R bcast
            nc.vector.tensor_copy(ones_col[:], ones_f[:, 0:P])
            eb_sb = consts.tile([P, NTC], F32)  # exp(memory_bias), col=tc
            nc.gpsimd.dma_start(eb_sb[:], ebd[:])

            # ---- persistent activations ----
            # qT/kT: per head-pair tile [128 (2 heads x 64 dh), L]
            qTp = [persist.tile([P, LQ], MM, name=f"qTp{i}") for i in range(2)]
            kTp = [persist.tile([P, LM], MM, name=f"kTp{i}") for i in range(2)]
            # v: per t-chunk [128 t, 4 heads x (64 v cols + e^b col + pad)]
            v_sb = [vpool.tile([P, HPC * VS], MM, name=f"v{t}")
                    for t in range(NTC)]
            wop = [persist.tile([P, D], MM, name=f"wop{i}")
                   for i in range(2)]

            # ======= fused projections + attention, software-pipelined =======
            # Emission order (per-engine streams are in-order, so emission
            # order is the schedule):
            #   rounds 0..3:  k/v projections for t-window r, q projection
            #                 for f-block 0 (round 0 only), then fb0's logits
            #                 groups {2r, 2r+1} for both head pairs
            #   fb blocks 1..3: q projection, then 16 logits+exp steps, with
            #                 AV matmuls of previously-completed (fb, hp)
            #                 blocks drained from a FIFO at 4 per step
            #   tail:         remaining AV units + last out-projection
            # PSUM: psl 3x[128, G*FB] (logits pairs + proj/rb/out-proj
            # rotations) + ppv 1 (v, phase 1 only) + psx 2 (AV accumulators).
            wq_sb = persist.tile([P, KD * JC], MM, name="wq")
            bq_sb = persist.tile([1, JC], MM, name="bq")
            wk_sb = persist.tile([P, KD * JC], MM, name="wk")
            wv_sb = persist.tile([P, KD * JC], MM, name="wv")
            bk_sb = persist.tile([1, JC], MM, name="bk")
            bv_sb = persist.tile([1, JC], MM, name="bv")

            with (
                tc.tile_pool(name="mrhs", bufs=2) as mrhsp,
                tc.tile_pool(name="expp", bufs=34) as expp,
                tc.tile_pool(name="attnp", bufs=6) as attnp,
                tc.tile_pool(name="rp", bufs=6) as rp,
                tc.tile_pool(name="rbp", bufs=4) as rbp,
                tc.tile_pool(name="osb", bufs=4) as osb,
                tc.tile_pool(name="psl", bufs=3, space="PSUM") as psl,
            ):
                for w_sb, wd in ((wk_sb, wkd), (wv_sb, wvd)):
                    h_ = KD // 2
                    nc.sync.dma_start(
                        w_sb[:, 0:h_ * JC].rearrange("p (k j) -> p k j", k=h_),
                        wd[ds(0, h_ * P), :].rearrange("(k p) j -> p k j", p=P))
                    nc.gpsimd.dma_start(
                        w_sb[:, h_ * JC:].rearrange("p (k j) -> p k j", k=h_),
                        wd[ds(h_ * P, h_ * P), :].rearrange(
                            "(k p) j -> p k j", p=P))
                if with_biases:
                    nc.gpsimd.dma_start(bk_sb[:], bkd[:])
                    nc.gpsimd.dma_start(bv_sb[:], bvd[:])
                nc.gpsimd.dma_start(
                    wq_sb[:].rearrange("p (k j) -> p k j", k=KD),
                    wqd.rearrange("(k p) j -> p k j", p=P))
                if with_biases:
                    nc.gpsimd.dma_start(bq_sb[:], bqd[:])
                for i in range(2):
                    nc.gpsimd.dma_start(wop[i][:], wod[ds(i * P, P), :])

                exps = {}      # (fb, hp, h2, g) -> [128, G*FB] bf16 tile
                apairs = {}    # (fb, hp) -> [128, FB] attn pair tile
                work_q = []    # FIFO: ["av", fb, hp, h2, tc_next, av_ap]
                               #       ["op", fb, fc]
                avail = {}     # (fb, hp) -> highest t-chunk with exp emitted

                def kq_proj(w_sb, b_sb, chunks, dstp, col):
                    for hp in range(2):
                        ps = psl.tile([P, G * FB], F32, name="pp", tag="pls")
                        for k in range(KD):
                            nc.tensor.matmul(
                                ps[:, 0:FB],
                                w_sb[:, ds(k * JC + hp * P, P)], chunks[k],
                                start=(k == 0),
                                stop=(not with_biases and k == KD - 1))
                        if with_biases:
                            nc.tensor.matmul(
                                ps[:, 0:FB], b_sb[:, ds(hp * P, P)],
                                ones_row[:], start=False, stop=True)
                        nc.vector.tensor_copy(dstp[hp][:, col], ps[:, 0:FB])

                def q_proj(fb):
                    qt = mrhsp.tile([P, KD * FB], MM, name="qchunk")
                    for k in range(KD):
                        eng = nc.sync if k % 2 == 0 else nc.gpsimd
                        eng.dma_start(qt[:, ts(k, FB)],
                                      qTd[ds(k * P, P), ts(fb, FB)])
                    kq_proj(wq_sb, bq_sb, [qt[:, ts(k, FB)] for k in range(KD)],
                            qTp, ts(fb, FB))

                def finish_unit(u):
                    _, fb, hp, h2, _, av = u
                    dn = rp.tile([1, FB], F32, name="dn")
                    nc.vector.tensor_copy(dn[:], av[ds(DH, 1), :])
                    rf = rp.tile([1, FB], F32, name="rf")
                    nc.vector.reciprocal_approx_fast(rf[:], dn[:])
                    rrow = rp.tile([1, FB], MM, name="rrow")
                    nc.vector.tensor_copy(rrow[:], rf[:])
                    rb_ps = psl.tile([P, G * FB], F32, name="rbps", tag="pls")
                    nc.tensor.matmul(rb_ps[:, 0:FB], ones_col[:], rrow[:],
                                     start=True, stop=True)
                    rb = rbp.tile([DH, FB], F32, name="rb")
                    nc.vector.tensor_copy(rb[:], rb_ps[0:DH, 0:FB])
                    if (fb, hp) not in apairs:
                        apairs[(fb, hp)] = attnp.tile([P, FB], MM,
                                                      name="apair")
                    nc.vector.tensor_tensor(
                        apairs[(fb, hp)][ds(h2 * DH, DH), :],
                        av[0:DH, :], rb[:], op=mybir.AluOpType.mult)
                    if hp == 1 and h2 == 1:
                        for fc in range(4):
                            work_q.append(["op", fb, fc])

                def out_proj_piece(fb, fc):
                    attn = [apairs[(fb, 0)], apairs[(fb, 1)]]
                    o = osb.tile([P, D], F32, name="osb")
                    for jb in range(2):
                        ops = psl.tile([P, G * FB], F32, name="ops",
                                       tag="pls")
                        for hp in range(2):
                            nc.tensor.matmul(
                                ops[:, 0:FB],
                                attn[hp][:, ds(fc * P, P)],
                                wop[hp][:, ts(jb, FB)],
                                start=(hp == 0), stop=(hp == 1))
                        nc.vector.tensor_copy(o[:, ts(jb, FB)],
                                              ops[:, 0:FB])
                    eng = nc.gpsimd if fc % 2 == 0 else nc.sync
                    eng.dma_start(outd[ds(fb * FB + fc * P, P), :], o[:])
                    if fc == 3:
                        apairs.pop((fb, 0))
                        apairs.pop((fb, 1))

                def drain_av(budget):
                    while budget > 0 and work_q:
                        u = work_q[0]
                        if u[0] == "op":
                            out_proj_piece(u[1], u[2])
                            work_q.pop(0)
                            budget -= 3
                            continue
                        _, fb, hp, h2, tcn, av = u
                        if tcn >= avail.get((fb, hp), 0):
                            break   # strict FIFO; head not yet runnable
                        h = 2 * hp + h2
                        nc.tensor.matmul(
                            av[:], v_sb[tcn][:, ds(h * VS, VW)],
                            exps[(fb, hp, h2, tcn // G)][:, ts(tcn % G, FB)],
                            start=(tcn == 0), stop=(tcn == NTC - 1))
                        u[4] += 1
                        budget -= 1
                        if u[4] == NTC:
                            work_q.pop(0)
                            finish_unit(u)

                def logits_step(fb, hp, g, av_budget=4):
                    pls = [psl.tile([P, G * FB], F32, name="pls")
                           for _ in range(2)]
                    for s in range(G):
                        t = g * G + s
                        for h2 in range(2):
                            nc.tensor.matmul(
                                pls[h2][:, ts(s, FB)],
                                kTp[hp][ds(h2 * DH, DH), ts(t, P)],
                                qTp[hp][ds(h2 * DH, DH), ts(fb, FB)],
                                start=True, stop=True)
                    for h2 in range(2):
                        e = expp.tile([P, G * FB], MM, name="exps")
                        nc.scalar.activation(e[:], pls[h2][:],
                                             mybir.ActivationFunctionType.Exp)
                        exps[(fb, hp, h2, g)] = e
                    avail[(fb, hp)] = (g + 1) * G
                    drain_av(av_budget)

                def enqueue_block(fb, hp):
                    for h2 in range(2):
                        av = psx.tile([P, FB], F32, name="av")[0:VW, :]
                        work_q.append(["av", fb, hp, h2, 0, av])

                # ---- phase 1: k/v rounds + fb0 logits ----
                with tc.tile_pool(name="ppv", bufs=1, space="PSUM") as ppv:
                    for rnd in range(NFB):
                        mt = mrhsp.tile([P, KD * FB], MM, name="mchunk")
                        for k in range(KD):
                            eng = nc.sync if k % 2 == 0 else nc.gpsimd
                            eng.dma_start(mt[:, ts(k, FB)],
                                          mTd[ds(k * P, P), ts(rnd, FB)])
                        chunks = [mt[:, ts(k, FB)] for k in range(KD)]
                        kq_proj(wk_sb, bk_sb, chunks, kTp, ts(rnd, FB))
                        if rnd == 0:
                            q_proj(0)
                        for g in (2 * rnd, 2 * rnd + 1):
                            for hp in range(2):
                                logits_step(0, hp, g)
                        for s in range(4):
                            t = rnd * 4 + s
                            psv = ppv.tile([P, JC], F32)
                            for k in range(KD):
                                nc.tensor.matmul(
                                    psv[:], chunks[k][:, ds(s * P, P)],
                                    wv_sb[:, ts(k, JC)],
                                    start=(k == 0),
                                    stop=(not with_biases and k == KD - 1))
                            if with_biases:
                                nc.tensor.matmul(
                                    psv[:], ones_col[:], bv_sb[:],
                                    start=False, stop=True)
                            dst = v_sb[t].rearrange("p (h c) -> p h c", h=HPC)
                            nc.vector.tensor_scalar_mul(
                                dst[:, :, 0:DH],
                                psv[:].rearrange("p (h c) -> p h c", h=HPC),
                                eb_sb[:, ds(t, 1)])
                            for h in range(HPC):
                                nc.vector.tensor_copy(
                                    dst[:, ds(h, 1), ds(DH, 1)],
                                    eb_sb[:, ds(t, 1)])

                # ---- steady state: fb blocks with AV drained in-stream ----
                with tc.tile_pool(name="psx", bufs=2, space="PSUM") as psx:
                    for hp in range(2):
                        enqueue_block(0, hp)
                    q_proj(1)
                    for fb in range(1, NFB):
                        for hp in range(2):
                            enqueue_block(fb, hp)
                            for g in range(NG):
                                logits_step(fb, hp, g,
                                            av_budget=5 if fb < 3 else 8)
                            if hp == 0 and fb < NFB - 1:
                                q_proj(fb + 1)
                    drain_av(10 ** 9)

    nc.compile()
    return nc


_CACHE = {}


def _get_module(with_biases=False):
    key = ("nc", with_biases)
    if key not in _CACHE:
        _CACHE[key] = build_kernel(with_biases=with_biases)
    return _CACHE[key]


def make_in_maps(query, memory, memory_bias, Wq, bq, Wk, bk, Wv, bv, Wo, bo,
                 mm_np=None, with_biases=False):
    if mm_np is None:
        import ml_dtypes
        mm_np = ml_dtypes.bfloat16
    query = np.asarray(query, np.float32)
    memory = np.asarray(memory, np.float32)
    memory_bias = np.asarray(memory_bias, np.float32)
    Wq = np.asarray(Wq, np.float32)
    bq = np.asarray(bq, np.float32)
    Wk = np.asarray(Wk, np.float32)
    bk = np.asarray(bk, np.float32)
    Wv = np.asarray(Wv, np.float32)
    bv = np.asarray(bv, np.float32)
    Wo = np.asarray(Wo, np.float32)
    s = np.float32(DH ** -0.5)

    qT = [np.ascontiguousarray(query[b].T).astype(mm_np) for b in range(B)]
    mT = [np.ascontiguousarray(memory[b].T).astype(mm_np) for b in range(B)]
    in_maps = []
    for c in range(NCORE):
        b, g = divmod(c, 4)
        J = slice(g * JC, (g + 1) * JC)
        m = {
            "qT": qT[b],
            "mT": mT[b],
            "wq": (np.ascontiguousarray(Wq[:, J]) * s).astype(mm_np),
            "wk": np.ascontiguousarray(Wk[:, J]).astype(mm_np),
            "wv": np.ascontiguousarray(Wv[:, J]).astype(mm_np),
            "wo": np.ascontiguousarray(Wo[J, :]).astype(mm_np),
            "eb": np.ascontiguousarray(
                np.exp(memory_bias[b].astype(np.float64)).reshape(
                    NTC, P).T).astype(np.float32),
        }
        if with_biases:
            m["bq"] = (bq[J] * s).reshape(1, JC).astype(mm_np)
            m["bk"] = bk[J].reshape(1, JC).astype(mm_np)
            m["bv"] = bv[J].reshape(1, JC).astype(mm_np)
        in_maps.append(m)
    return in_maps


def gather_output(results, bo):
    bo = np.asarray(bo, np.float32)
    out = np.empty((B, LQ, D), np.float32)
    for b in range(B):
        acc = results[4 * b]["out"].astype(np.float32)
        for g in range(1, 4):
            acc = acc + results[4 * b + g]["out"]
        out[b] = acc + bo
    return out


def kernel(**inputs):
    wb = any(np.any(np.asarray(inputs[b])) for b in ("bq", "bk", "bv"))
    nc = _get_module(with_biases=wb)
    in_maps = make_in_maps(**inputs, with_biases=wb)
    res = bass_utils.run_bass_kernel_spmd(nc, in_maps,
                                          core_ids=list(range(NCORE)))
    return gather_output(res.results, inputs["bo"])


# revision 33
# speedup vs baseline: 1.1707x; 1.0112x over previous
"""Multi-head attention (B=2, L=2048, D=1024, H=16) on 8 TRN2 NeuronCores.

Sharding: batch (2) x head-group (4 heads each) = 8 shards.
Each core computes q/k/v projections for its 4 heads, attention, and a
partial output projection (its 256 rows of Wo); host sums the 4 partials
per batch and adds bo.

Device dataflow (per core):
  inputs (host-prepped, bf16 except memory_bias):
    qT  [1024, 2048]  = query[b].T          (d on partitions for matmul)
    mT  [1024, 2048]  = memory[b].T
    wq  [1024, 256]   = Wq[:, J] * 0.125    (scale folded)
    wk, wv [1024, 256];  bq*0.125, bk, bv [1, 256]
    wo  [256, 1024]   = Wo[J, :]
    mb  [16, 128]     = memory_bias[b]  (f32)
  phase 1: qT_h [j, f], kT_h [j, t] (head-dim on partitions), v [t, j]
           (natural), biases folded in via K=1 ones matmuls, and
           v scaled by exp(memory_bias[t]) so the softmax bias drops out:
           softmax(s + b) @ v == (exp(s) @ (v * e^b)) / (exp(s) @ e^b)
  phase 2: sT = kT.T@qT (two heads row-tiled to overlap in the PE array)
           -> exp on ScalarE (the phase-2 bottleneck, kept saturated via
           a triple-buffered psum pool) -> AV matmul with an extra
           e^b column producing the softmax denominator row for free ->
           normalize into a [128, f] head-pair tile -> row-paired output
           projection partial [f, j].
"""

import numpy as np

import concourse.bass as bass
import concourse.tile as tile
from concourse import bacc, mybir
from concourse import bass_utils
from concourse.bass import ts, ds

F32 = mybir.dt.float32
F32R = mybir.dt.float32r
BF16 = mybir.dt.bfloat16

B, LQ, LM, D, H = 2, 2048, 2048, 1024, 16
DH = 64
HPC = 4            # heads per core
JC = HPC * DH      # 256 projection cols per core
NCORE = 8
P = 128
FB = 512           # f-block width
NFB = LQ // FB     # 4
NTC = LM // P      # 16 t-chunks
KD = D // P        # 8 contraction chunks for projections
G = 2              # t-chunks per exp group (psum tile [128, G*512])
NG = NTC // G      # 8 groups

VW = DH + 1        # v columns per head incl. denominator column
VS = DH + 2        # v column stride per head (4B alignment in bf16)


def build_kernel(mm_dt=BF16, with_biases=False):
    MM = mm_dt
    nc = bacc.Bacc("TRN2", target_bir_lowering=False, debug=False)

    qTd = nc.dram_tensor("qT", [D, LQ], MM, kind="ExternalInput").ap()
    mTd = nc.dram_tensor("mT", [D, LM], MM, kind="ExternalInput").ap()
    wqd = nc.dram_tensor("wq", [D, JC], MM, kind="ExternalInput").ap()
    wkd = nc.dram_tensor("wk", [D, JC], MM, kind="ExternalInput").ap()
    wvd = nc.dram_tensor("wv", [D, JC], MM, kind="ExternalInput").ap()
    if with_biases:
        bqd = nc.dram_tensor("bq", [1, JC], MM, kind="ExternalInput").ap()
        bkd = nc.dram_tensor("bk", [1, JC], MM, kind="ExternalInput").ap()
        bvd = nc.dram_tensor("bv", [1, JC], MM, kind="ExternalInput").ap()
    wod = nc.dram_tensor("wo", [JC, D], MM, kind="ExternalInput").ap()
    ebd = nc.dram_tensor("eb", [P, NTC], F32, kind="ExternalInput").ap()
    outd = nc.dram_tensor("out", [LQ, D], F32, kind="ExternalOutput").ap()

    with tile.TileContext(nc) as tc:
        with (
            tc.tile_pool(name="persist", bufs=1) as persist,
            tc.tile_pool(name="vpool", bufs=1) as vpool,
            tc.tile_pool(name="consts", bufs=1) as consts,
        ):
            # ---- constants ----
            ones_f = consts.tile([1, FB], F32)
            nc.vector.memset(ones_f[:], 1.0)
            ones_row = consts.tile([1, FB], MM)      # rhs for bias matmuls
            nc.vector.tensor_copy(ones_row[:], ones_f[:])
            ones_col = consts.tile([1, P], MM)       # lhsT for v-bias / R bcast
            nc.vector.tensor_copy(ones_col[:], ones_f[:, 0:P])
            eb_sb = consts.tile([P, NTC], F32)  # exp(memory_bias), col=tc
            nc.gpsimd.dma_start(eb_sb[:], ebd[:])

            # ---- persistent activations ----
            # qT/kT: per head-pair tile [128 (2 heads x 64 dh), L]
            qTp = [persist.tile([P, LQ], MM, name=f"qTp{i}") for i in range(2)]
            kTp = [persist.tile([P, LM], MM, name=f"kTp{i}") for i in range(2)]
            # v: per t-chunk [128 t, 4 heads x (64 v cols + e^b col + pad)]
            v_sb = [vpool.tile([P, HPC * VS], MM, name=f"v{t}")
                    for t in range(NTC)]
            wop = [persist.tile([P, D], MM, name=f"wop{i}")
                   for i in range(2)]

            # ======= fused projections + attention, software-pipelined =======
            # Emission order (per-engine streams are in-order, so emission
            # order is the schedule):
            #   rounds 0..3:  k/v projections for t-window r, q projection
            #                 for f-block 0 (round 0 only), then fb0's logits
            #                 groups {2r, 2r+1} for both head pairs
            #   fb blocks 1..3: q projection, then 16 logits+exp steps, with
            #                 AV matmuls of previously-completed (fb, hp)
            #                 blocks drained from a FIFO at 4 per step
            #   tail:         remaining AV units + last out-projection
            # PSUM: psl 3x[128, G*FB] (logits pairs + proj/rb/out-proj
            # rotations) + ppv 1 (v, phase 1 only) + psx 2 (AV accumulators).
            wq_sb = persist.tile([P, KD * JC], MM, name="wq")
            bq_sb = persist.tile([1, JC], MM, name="bq")
            wk_sb = persist.tile([P, KD * JC], MM, name="wk")
            wv_sb = persist.tile([P, KD * JC], MM, name="wv")
            bk_sb = persist.tile([1, JC], MM, name="bk")
            bv_sb = persist.tile([1, JC], MM, name="bv")

            with (
                tc.tile_pool(name="mrhs", bufs=2) as mrhsp,
                tc.tile_pool(name="expp", bufs=34) as expp,
                tc.tile_pool(name="attnp", bufs=6) as attnp,
                tc.tile_pool(name="rp", bufs=6) as rp,
                tc.tile_pool(name="rbp", bufs=4) as rbp,
                tc.tile_pool(name="osb", bufs=4) as osb,
                tc.tile_pool(name="psl", bufs=3, space="PSUM") as psl,
            ):
                for w_sb, wd in ((wk_sb, wkd), (wv_sb, wvd)):
                    h_ = KD // 2
                    nc.sync.dma_start(
                        w_sb[:, 0:h_ * JC].rearrange("p (k j) -> p k j", k=h_),
                        wd[ds(0, h_ * P), :].rearrange("(k p) j -> p k j", p=P))
                    nc.gpsimd.dma_start(
                        w_sb[:, h_ * JC:].rearrange("p (k j) -> p k j", k=h_),
                        wd[ds(h_ * P, h_ * P), :].rearrange(
                            "(k p) j -> p k j", p=P))
                if with_biases:
                    nc.gpsimd.dma_start(bk_sb[:], bkd[:])
                    nc.gpsimd.dma_start(bv_sb[:], bvd[:])
                nc.gpsimd.dma_start(
                    wq_sb[:].rearrange("p (k j) -> p k j", k=KD),
                    wqd.rearrange("(k p) j -> p k j", p=P))
                if with_biases:
                    nc.gpsimd.dma_start(bq_sb[:], bqd[:])
                for i in range(2):
                    nc.gpsimd.dma_start(wop[i][:], wod[ds(i * P, P), :])

                exps = {}      # (fb, hp, h2, g) -> [128, G*FB] bf16 tile
                apairs = {}    # (fb, hp) -> [128, FB] attn pair tile
                work_q = []    # FIFO: ["av", fb, hp, h2, tc_next, av_ap]
                               #       ["op", fb, fc]
                avail = {}     # (fb, hp) -> highest t-chunk with exp emitted

                def kq_proj(w_sb, b_sb, chunks, dstp, col):
                    for hp in range(2):
                        ps = psl.tile([P, G * FB], F32, name="pp", tag="pls")
                        for k in range(KD):
                            nc.tensor.matmul(
                                ps[:, 0:FB],
                                w_sb[:, ds(k * JC + hp * P, P)], chunks[k],
                                start=(k == 0),
                                stop=(not with_biases and k == KD - 1))
                        if with_biases:
                            nc.tensor.matmul(
                                ps[:, 0:FB], b_sb[:, ds(hp * P, P)],
                                ones_row[:], start=False, stop=True)
                        nc.vector.tensor_copy(dstp[hp][:, col], ps[:, 0:FB])

                def q_proj(fb):
                    qt = mrhsp.tile([P, KD * FB], MM, name="qchunk")
                    for k in range(KD):
                        eng = nc.sync if k % 2 == 0 else nc.gpsimd
                        eng.dma_start(qt[:, ts(k, FB)],
                                      qTd[ds(k * P, P), ts(fb, FB)])
                    kq_proj(wq_sb, bq_sb, [qt[:, ts(k, FB)] for k in range(KD)],
                            qTp, ts(fb, FB))

                def finish_unit(u):
                    _, fb, hp, h2, _, av = u
                    dn = rp.tile([1, FB], F32, name="dn")
                    nc.vector.tensor_copy(dn[:], av[ds(DH, 1), :])
                    rf = rp.tile([1, FB], F32, name="rf")
                    nc.vector.reciprocal_approx_fast(rf[:], dn[:])
                    rrow = rp.tile([1, FB], MM, name="rrow")
                    nc.vector.tensor_copy(rrow[:], rf[:])
                    rb_ps = psl.tile([P, G * FB], F32, name="rbps", tag="pls")
                    nc.tensor.matmul(rb_ps[:, 0:FB], ones_col[:], rrow[:],
                                     start=True, stop=True)
                    rb = rbp.tile([DH, FB], F32, name="rb")
                    nc.vector.tensor_copy(rb[:], rb_ps[0:DH, 0:FB])
                    if (fb, hp) not in apairs:
                        apairs[(fb, hp)] = attnp.tile([P, FB], MM,
                                                      name="apair")
                    nc.vector.tensor_tensor(
                        apairs[(fb, hp)][ds(h2 * DH, DH), :],
                        av[0:DH, :], rb[:], op=mybir.AluOpType.mult)
                    if hp == 1 and h2 == 1:
                        for fc in range(4):
                            work_q.append(["op", fb, fc])

                def out_proj_piece(fb, fc):
                    attn = [apairs[(fb, 0)], apairs[(fb, 1)]]
                    o = osb.tile([P, D], F32, name="osb")
                    for jb in range(2):
                        ops = psl.tile([P, G * FB], F32, name="ops",
                                       tag="pls")
                        for hp in range(2):
                            nc.tensor.matmul(
                                ops[:, 0:FB],
                                attn[hp][:, ds(fc * P, P)],
                                wop[hp][:, ts(jb, FB)],
                                start=(hp == 0), stop=(hp == 1))
                        nc.vector.tensor_copy(o[:, ts(jb, FB)],
                                              ops[:, 0:FB])
                    eng = nc.gpsimd if fc % 2 == 0 else nc.sync
                    eng.dma_start(outd[ds(fb * FB + fc * P, P), :], o[:])
                    if fc == 3:
                        apairs.pop((fb, 0))
                        apairs.pop((fb, 1))

                def drain_av(budget):
                    while budget > 0 and work_q:
                        u = work_q[0]
                        if u[0] == "op":
                            out_proj_piece(u[1], u[2])
                            work_q.pop(0)
                            budget -= 3
                            continue
                        _, fb, hp, h2, tcn, av = u
                        if tcn >= avail.get((fb, hp), 0):
                            break   # strict FIFO; head not yet runnable
                        h = 2 * hp + h2
                        nc.tensor.matmul(
                            av[:], v_sb[tcn][:, ds(h * VS, VW)],
                            exps[(fb, hp, h2, tcn // G)][:, ts(tcn % G, FB)],
                            start=(tcn == 0), stop=(tcn == NTC - 1))
                        u[4] += 1
                        budget -= 1
                        if u[4] == NTC:
                            work_q.pop(0)
                            finish_unit(u)

                def logits_step(fb, hp, g, av_budget=4):
                    pls = [psl.tile([P, G * FB], F32, name="pls")
                           for _ in range(2)]
                    for s in range(G):
                        t = g * G + s
                        for h2 in range(2):
                            nc.tensor.matmul(
                                pls[h2][:, ts(s, FB)],
                                kTp[hp][ds(h2 * DH, DH), ts(t, P)],
                                qTp[hp][ds(h2 * DH, DH), ts(fb, FB)],
                                start=True, stop=True)
                    for h2 in range(2):
                        e = expp.tile([P, G * FB], MM, name="exps")
                        nc.scalar.activation(e[:], pls[h2][:],
                                             mybir.ActivationFunctionType.Exp)
                        exps[(fb, hp, h2, g)] = e
                    avail[(fb, hp)] = (g + 1) * G
                    drain_av(av_budget)

                def enqueue_block(fb, hp):
                    for h2 in range(2):
                        av = psx.tile([P, FB], F32, name="av")[0:VW, :]
                        work_q.append(["av", fb, hp, h2, 0, av])

                # ---- phase 1: k/v rounds + fb0 logits ----
                with tc.tile_pool(name="ppv", bufs=1, space="PSUM") as ppv:
                    for rnd in range(NFB):
                        mt = mrhsp.tile([P, KD * FB], MM, name="mchunk")
                        for k in range(KD):
                            eng = nc.sync if k % 2 == 0 else nc.gpsimd
                            eng.dma_start(mt[:, ts(k, FB)],
                                          mTd[ds(k * P, P), ts(rnd, FB)])
                        chunks = [mt[:, ts(k, FB)] for k in range(KD)]
                        kq_proj(wk_sb, bk_sb, chunks, kTp, ts(rnd, FB))
                        if rnd == 0:
                            q_proj(0)
                        lsteps = [(g, hp) for g in (2 * rnd, 2 * rnd + 1)
                                  for hp in range(2)]
                        for s in range(4):
                            g, hp = lsteps[s]
                            logits_step(0, hp, g)
                            t = rnd * 4 + s
                            psv = ppv.tile([P, JC], F32)
                            for k in range(KD):
                                nc.tensor.matmul(
                                    psv[:], chunks[k][:, ds(s * P, P)],
                                    wv_sb[:, ts(k, JC)],
                                    start=(k == 0),
                                    stop=(not with_biases and k == KD - 1))
                            if with_biases:
                                nc.tensor.matmul(
                                    psv[:], ones_col[:], bv_sb[:],
                                    start=False, stop=True)
                            dst = v_sb[t].rearrange("p (h c) -> p h c", h=HPC)
                            nc.vector.tensor_scalar_mul(
                                dst[:, :, 0:DH],
                                psv[:].rearrange("p (h c) -> p h c", h=HPC),
                                eb_sb[:, ds(t, 1)])
                            for h in range(HPC):
                                nc.vector.tensor_copy(
                                    dst[:, ds(h, 1), ds(DH, 1)],
                                    eb_sb[:, ds(t, 1)])

                # ---- steady state: fb blocks with AV drained in-stream ----
                with tc.tile_pool(name="psx", bufs=2, space="PSUM") as psx:
                    for hp in range(2):
                        enqueue_block(0, hp)
                    q_proj(1)
                    for fb in range(1, NFB):
                        for hp in range(2):
                            enqueue_block(fb, hp)
                            for g in range(NG):
                                logits_step(fb, hp, g,
                                            av_budget=5 if fb < 3 else 8)
                            if hp == 0 and fb < NFB - 1:
                                q_proj(fb + 1)
                    drain_av(10 ** 9)

    nc.compile()
    return nc


_CACHE = {}


def _get_module(with_biases=False):
    key = ("nc", with_biases)
    if key not in _CACHE:
        _CACHE[key] = build_kernel(with_biases=with_biases)
    return _CACHE[key]


def make_in_maps(query, memory, memory_bias, Wq, bq, Wk, bk, Wv, bv, Wo, bo,
                 mm_np=None, with_biases=False):
    if mm_np is None:
        import ml_dtypes
        mm_np = ml_dtypes.bfloat16
    query = np.asarray(query, np.float32)
    memory = np.asarray(memory, np.float32)
    memory_bias = np.asarray(memory_bias, np.float32)
    Wq = np.asarray(Wq, np.float32)
    bq = np.asarray(bq, np.float32)
    Wk = np.asarray(Wk, np.float32)
    bk = np.asarray(bk, np.float32)
    Wv = np.asarray(Wv, np.float32)
    bv = np.asarray(bv, np.float32)
    Wo = np.asarray(Wo, np.float32)
    s = np.float32(DH ** -0.5)

    qT = [np.ascontiguousarray(query[b].T).astype(mm_np) for b in range(B)]
    mT = [np.ascontiguousarray(memory[b].T).astype(mm_np) for b in range(B)]
    in_maps = []
    for c in range(NCORE):
        b, g = divmod(c, 4)
        J = slice(g * JC, (g + 1) * JC)
        m = {
            "qT": qT[b],
            "mT": mT[b],
            "wq": (np.ascontiguousarray(Wq[:, J]) * s).astype(mm_np),
            "wk": np.ascontiguousarray(Wk[:, J]).astype(mm_np),
            "wv": np.ascontiguousarray(Wv[:, J]).astype(mm_np),
            "wo": np.ascontiguousarray(Wo[J, :]).astype(mm_np),
            "eb": np.ascontiguousarray(
                np.exp(memory_bias[b].astype(np.float64)).reshape(
                    NTC, P).T).astype(np.float32),
        }
        if with_biases:
            m["bq"] = (bq[J] * s).reshape(1, JC).astype(mm_np)
            m["bk"] = bk[J].reshape(1, JC).astype(mm_np)
            m["bv"] = bv[J].reshape(1, JC).astype(mm_np)
        in_maps.append(m)
    return in_maps


def gather_output(results, bo):
    bo = np.asarray(bo, np.float32)
    out = np.empty((B, LQ, D), np.float32)
    for b in range(B):
        acc = results[4 * b]["out"].astype(np.float32)
        for g in range(1, 4):
            acc = acc + results[4 * b + g]["out"]
        out[b] = acc + bo
    return out


def kernel(**inputs):
    wb = any(np.any(np.asarray(inputs[b])) for b in ("bq", "bk", "bv"))
    nc = _get_module(with_biases=wb)
    in_maps = make_in_maps(**inputs, with_biases=wb)
    res = bass_utils.run_bass_kernel_spmd(nc, in_maps,
                                          core_ids=list(range(NCORE)))
    return gather_output(res.results, inputs["bo"])


# revision 34
# speedup vs baseline: 1.1888x; 1.0154x over previous
"""Multi-head attention (B=2, L=2048, D=1024, H=16) on 8 TRN2 NeuronCores.

Sharding: batch (2) x head-group (4 heads each) = 8 shards.
Each core computes q/k/v projections for its 4 heads, attention, and a
partial output projection (its 256 rows of Wo); host sums the 4 partials
per batch and adds bo.

Device dataflow (per core):
  inputs (host-prepped, bf16 except memory_bias):
    qT  [1024, 2048]  = query[b].T          (d on partitions for matmul)
    mT  [1024, 2048]  = memory[b].T
    wq  [1024, 256]   = Wq[:, J] * 0.125    (scale folded)
    wk, wv [1024, 256];  bq*0.125, bk, bv [1, 256]
    wo  [256, 1024]   = Wo[J, :]
    mb  [16, 128]     = memory_bias[b]  (f32)
  phase 1: qT_h [j, f], kT_h [j, t] (head-dim on partitions), v [t, j]
           (natural), biases folded in via K=1 ones matmuls, and
           v scaled by exp(memory_bias[t]) so the softmax bias drops out:
           softmax(s + b) @ v == (exp(s) @ (v * e^b)) / (exp(s) @ e^b)
  phase 2: sT = kT.T@qT (two heads row-tiled to overlap in the PE array)
           -> exp on ScalarE (the phase-2 bottleneck, kept saturated via
           a triple-buffered psum pool) -> AV matmul with an extra
           e^b column producing the softmax denominator row for free ->
           normalize into a [128, f] head-pair tile -> row-paired output
           projection partial [f, j].
"""

import numpy as np

import concourse.bass as bass
import concourse.tile as tile
from concourse import bacc, mybir
from concourse import bass_utils
from concourse.bass import ts, ds

F32 = mybir.dt.float32
F32R = mybir.dt.float32r
BF16 = mybir.dt.bfloat16

B, LQ, LM, D, H = 2, 2048, 2048, 1024, 16
DH = 64
HPC = 4            # heads per core
JC = HPC * DH      # 256 projection cols per core
NCORE = 8
P = 128
FB = 512           # f-block width
NFB = LQ // FB     # 4
NTC = LM // P      # 16 t-chunks
KD = D // P        # 8 contraction chunks for projections
G = 2              # t-chunks per exp group (psum tile [128, G*512])
NG = NTC // G      # 8 groups

VW = DH + 1        # v columns per head incl. denominator column
VS = DH + 2        # v column stride per head (4B alignment in bf16)


def build_kernel(mm_dt=BF16, with_biases=False):
    MM = mm_dt
    nc = bacc.Bacc("TRN2", target_bir_lowering=False, debug=False)

    qTd = nc.dram_tensor("qT", [D, LQ], MM, kind="ExternalInput").ap()
    mTd = nc.dram_tensor("mT", [D, LM], MM, kind="ExternalInput").ap()
    wqd = nc.dram_tensor("wq", [D, JC], MM, kind="ExternalInput").ap()
    wkd = nc.dram_tensor("wk", [D, JC], MM, kind="ExternalInput").ap()
    wvd = nc.dram_tensor("wv", [D, JC], MM, kind="ExternalInput").ap()
    if with_biases:
        bqd = nc.dram_tensor("bq", [1, JC], MM, kind="ExternalInput").ap()
        bkd = nc.dram_tensor("bk", [1, JC], MM, kind="ExternalInput").ap()
        bvd = nc.dram_tensor("bv", [1, JC], MM, kind="ExternalInput").ap()
    wod = nc.dram_tensor("wo", [JC, D], MM, kind="ExternalInput").ap()
    ebd = nc.dram_tensor("eb", [P, NTC], F32, kind="ExternalInput").ap()
    outd = nc.dram_tensor("out", [LQ, D], F32, kind="ExternalOutput").ap()

    with tile.TileContext(nc) as tc:
        with (
            tc.tile_pool(name="persist", bufs=1) as persist,
            tc.tile_pool(name="vpool", bufs=1) as vpool,
            tc.tile_pool(name="consts", bufs=1) as consts,
        ):
            # ---- constants ----
            ones_f = consts.tile([1, FB], F32)
            nc.vector.memset(ones_f[:], 1.0)
            ones_row = consts.tile([1, FB], MM)      # rhs for bias matmuls
            nc.vector.tensor_copy(ones_row[:], ones_f[:])
            ones_col = consts.tile([1, P], MM)       # lhsT for v-bias / R bcast
            nc.vector.tensor_copy(ones_col[:], ones_f[:, 0:P])
            eb_sb = consts.tile([P, NTC], F32)  # exp(memory_bias), col=tc
            nc.gpsimd.dma_start(eb_sb[:], ebd[:])

            # ---- persistent activations ----
            # qT/kT: per head-pair tile [128 (2 heads x 64 dh), L]
            qTp = [persist.tile([P, LQ], MM, name=f"qTp{i}") for i in range(2)]
            kTp = [persist.tile([P, LM], MM, name=f"kTp{i}") for i in range(2)]
            # v: per t-chunk [128 t, 4 heads x (64 v cols + e^b col + pad)]
            v_sb = [vpool.tile([P, HPC * VS], MM, name=f"v{t}")
                    for t in range(NTC)]
            wop = [persist.tile([P, D], MM, name=f"wop{i}")
                   for i in range(2)]

            # ======= fused projections + attention, software-pipelined =======
            # Emission order (per-engine streams are in-order, so emission
            # order is the schedule):
            #   rounds 0..3:  k/v projections for t-window r, q projection
            #                 for f-block 0 (round 0 only), then fb0's logits
            #                 groups {2r, 2r+1} for both head pairs
            #   fb blocks 1..3: q projection, then 16 logits+exp steps, with
            #                 AV matmuls of previously-completed (fb, hp)
            #                 blocks drained from a FIFO at 4 per step
            #   tail:         remaining AV units + last out-projection
            # PSUM: psl 3x[128, G*FB] (logits pairs + proj/rb/out-proj
            # rotations) + ppv 1 (v, phase 1 only) + psx 2 (AV accumulators).
            wq_sb = persist.tile([P, KD * JC], MM, name="wq")
            bq_sb = persist.tile([1, JC], MM, name="bq")
            wk_sb = persist.tile([P, KD * JC], MM, name="wk")
            wv_sb = persist.tile([P, KD * JC], MM, name="wv")
            bk_sb = persist.tile([1, JC], MM, name="bk")
            bv_sb = persist.tile([1, JC], MM, name="bv")

            with (
                tc.tile_pool(name="mrhs", bufs=2) as mrhsp,
                tc.tile_pool(name="expp", bufs=34) as expp,
                tc.tile_pool(name="attnp", bufs=6) as attnp,
                tc.tile_pool(name="rp", bufs=6) as rp,
                tc.tile_pool(name="rbp", bufs=4) as rbp,
                tc.tile_pool(name="osb", bufs=4) as osb,
                tc.tile_pool(name="psl", bufs=3, space="PSUM") as psl,
            ):
                for w_sb, wd in ((wk_sb, wkd), (wv_sb, wvd)):
                    h_ = KD // 2
                    nc.sync.dma_start(
                        w_sb[:, 0:h_ * JC].rearrange("p (k j) -> p k j", k=h_),
                        wd[ds(0, h_ * P), :].rearrange("(k p) j -> p k j", p=P))
                    nc.gpsimd.dma_start(
                        w_sb[:, h_ * JC:].rearrange("p (k j) -> p k j", k=h_),
                        wd[ds(h_ * P, h_ * P), :].rearrange(
                            "(k p) j -> p k j", p=P))
                if with_biases:
                    nc.gpsimd.dma_start(bk_sb[:], bkd[:])
                    nc.gpsimd.dma_start(bv_sb[:], bvd[:])
                def late_weight_loads():
                    # issued after round 0's input chunks so the big wq/wo
                    # transfers don't block the first k-projection loads
                    h_ = KD // 2
                    nc.sync.dma_start(
                        wq_sb[:, 0:h_ * JC].rearrange("p (k j) -> p k j",
                                                      k=h_),
                        wqd[ds(0, h_ * P), :].rearrange("(k p) j -> p k j",
                                                        p=P))
                    nc.gpsimd.dma_start(
                        wq_sb[:, h_ * JC:].rearrange("p (k j) -> p k j",
                                                     k=h_),
                        wqd[ds(h_ * P, h_ * P), :].rearrange(
                            "(k p) j -> p k j", p=P))
                    if with_biases:
                        nc.gpsimd.dma_start(bq_sb[:], bqd[:])
                    for i in range(2):
                        nc.gpsimd.dma_start(wop[i][:], wod[ds(i * P, P), :])

                exps = {}      # (fb, hp, h2, g) -> [128, G*FB] bf16 tile
                apairs = {}    # (fb, hp) -> [128, FB] attn pair tile
                work_q = []    # FIFO: ["av", fb, hp, h2, tc_next, av_ap]
                               #       ["op", fb, fc]
                avail = {}     # (fb, hp) -> highest t-chunk with exp emitted

                def kq_proj(w_sb, b_sb, chunks, dstp, col):
                    for hp in range(2):
                        ps = psl.tile([P, G * FB], F32, name="pp", tag="pls")
                        for k in range(KD):
                            nc.tensor.matmul(
                                ps[:, 0:FB],
                                w_sb[:, ds(k * JC + hp * P, P)], chunks[k],
                                start=(k == 0),
                                stop=(not with_biases and k == KD - 1))
                        if with_biases:
                            nc.tensor.matmul(
                                ps[:, 0:FB], b_sb[:, ds(hp * P, P)],
                                ones_row[:], start=False, stop=True)
                        nc.vector.tensor_copy(dstp[hp][:, col], ps[:, 0:FB])

                def q_proj(fb):
                    qt = mrhsp.tile([P, KD * FB], MM, name="qchunk")
                    for k in range(KD):
                        eng = nc.sync if k % 2 == 0 else nc.gpsimd
                        eng.dma_start(qt[:, ts(k, FB)],
                                      qTd[ds(k * P, P), ts(fb, FB)])
                    kq_proj(wq_sb, bq_sb, [qt[:, ts(k, FB)] for k in range(KD)],
                            qTp, ts(fb, FB))

                def finish_unit(u):
                    _, fb, hp, h2, _, av = u
                    dn = rp.tile([1, FB], F32, name="dn")
                    nc.vector.tensor_copy(dn[:], av[ds(DH, 1), :])
                    rf = rp.tile([1, FB], F32, name="rf")
                    nc.vector.reciprocal_approx_fast(rf[:], dn[:])
                    rrow = rp.tile([1, FB], MM, name="rrow")
                    nc.vector.tensor_copy(rrow[:], rf[:])
                    rb_ps = psl.tile([P, G * FB], F32, name="rbps", tag="pls")
                    nc.tensor.matmul(rb_ps[:, 0:FB], ones_col[:], rrow[:],
                                     start=True, stop=True)
                    rb = rbp.tile([DH, FB], F32, name="rb")
                    nc.vector.tensor_copy(rb[:], rb_ps[0:DH, 0:FB])
                    if (fb, hp) not in apairs:
                        apairs[(fb, hp)] = attnp.tile([P, FB], MM,
                                                      name="apair")
                    nc.vector.tensor_tensor(
                        apairs[(fb, hp)][ds(h2 * DH, DH), :],
                        av[0:DH, :], rb[:], op=mybir.AluOpType.mult)
                    if hp == 1 and h2 == 1:
                        for fc in range(4):
                            work_q.append(["op", fb, fc])

                def out_proj_piece(fb, fc):
                    attn = [apairs[(fb, 0)], apairs[(fb, 1)]]
                    o = osb.tile([P, D], F32, name="osb")
                    for jb in range(2):
                        ops = psl.tile([P, G * FB], F32, name="ops",
                                       tag="pls")
                        for hp in range(2):
                            nc.tensor.matmul(
                                ops[:, 0:FB],
                                attn[hp][:, ds(fc * P, P)],
                                wop[hp][:, ts(jb, FB)],
                                start=(hp == 0), stop=(hp == 1))
                        nc.vector.tensor_copy(o[:, ts(jb, FB)],
                                              ops[:, 0:FB])
                    eng = nc.gpsimd if fc % 2 == 0 else nc.sync
                    eng.dma_start(outd[ds(fb * FB + fc * P, P), :], o[:])
                    if fc == 3:
                        apairs.pop((fb, 0))
                        apairs.pop((fb, 1))

                def drain_av(budget):
                    while budget > 0 and work_q:
                        u = work_q[0]
                        if u[0] == "op":
                            out_proj_piece(u[1], u[2])
                            work_q.pop(0)
                            budget -= 3
                            continue
                        _, fb, hp, h2, tcn, av = u
                        if tcn >= avail.get((fb, hp), 0):
                            break   # strict FIFO; head not yet runnable
                        h = 2 * hp + h2
                        nc.tensor.matmul(
                            av[:], v_sb[tcn][:, ds(h * VS, VW)],
                            exps[(fb, hp, h2, tcn // G)][:, ts(tcn % G, FB)],
                            start=(tcn == 0), stop=(tcn == NTC - 1))
                        u[4] += 1
                        budget -= 1
                        if u[4] == NTC:
                            work_q.pop(0)
                            finish_unit(u)

                def logits_step(fb, hp, g, av_budget=4):
                    pls = [psl.tile([P, G * FB], F32, name="pls")
                           for _ in range(2)]
                    for s in range(G):
                        t = g * G + s
                        for h2 in range(2):
                            nc.tensor.matmul(
                                pls[h2][:, ts(s, FB)],
                                kTp[hp][ds(h2 * DH, DH), ts(t, P)],
                                qTp[hp][ds(h2 * DH, DH), ts(fb, FB)],
                                start=True, stop=True)
                    for h2 in range(2):
                        e = expp.tile([P, G * FB], MM, name="exps")
                        nc.scalar.activation(e[:], pls[h2][:],
                                             mybir.ActivationFunctionType.Exp)
                        exps[(fb, hp, h2, g)] = e
                    avail[(fb, hp)] = (g + 1) * G
                    drain_av(av_budget)

                def enqueue_block(fb, hp):
                    for h2 in range(2):
                        av = psx.tile([P, FB], F32, name="av")[0:VW, :]
                        work_q.append(["av", fb, hp, h2, 0, av])

                # ---- phase 1: k/v rounds + fb0 logits ----
                with tc.tile_pool(name="ppv", bufs=1, space="PSUM") as ppv:
                    for rnd in range(NFB):
                        mt = mrhsp.tile([P, KD * FB], MM, name="mchunk")
                        for k in range(KD):
                            eng = nc.sync if k % 2 == 0 else nc.gpsimd
                            eng.dma_start(mt[:, ts(k, FB)],
                                          mTd[ds(k * P, P), ts(rnd, FB)])
                        chunks = [mt[:, ts(k, FB)] for k in range(KD)]
                        if rnd == 0:
                            late_weight_loads()
                        kq_proj(wk_sb, bk_sb, chunks, kTp, ts(rnd, FB))
                        if rnd == 0:
                            q_proj(0)
                        lsteps = [(g, hp) for g in (2 * rnd, 2 * rnd + 1)
                                  for hp in range(2)]
                        for s in range(4):
                            g, hp = lsteps[s]
                            logits_step(0, hp, g)
                            t = rnd * 4 + s
                            psv = ppv.tile([P, JC], F32)
                            for k in range(KD):
                                nc.tensor.matmul(
                                    psv[:], chunks[k][:, ds(s * P, P)],
                                    wv_sb[:, ts(k, JC)],
                                    start=(k == 0),
                                    stop=(not with_biases and k == KD - 1))
                            if with_biases:
                                nc.tensor.matmul(
                                    psv[:], ones_col[:], bv_sb[:],
                                    start=False, stop=True)
                            dst = v_sb[t].rearrange("p (h c) -> p h c", h=HPC)
                            nc.vector.tensor_scalar_mul(
                                dst[:, :, 0:DH],
                                psv[:].rearrange("p (h c) -> p h c", h=HPC),
                                eb_sb[:, ds(t, 1)])
                            for h in range(HPC):
                                nc.vector.tensor_copy(
                                    dst[:, ds(h, 1), ds(DH, 1)],
                                    eb_sb[:, ds(t, 1)])

                # ---- steady state: fb blocks with AV drained in-stream ----
                with tc.tile_pool(name="psx", bufs=2, space="PSUM") as psx:
                    for hp in range(2):
                        enqueue_block(0, hp)
                    q_proj(1)
                    for fb in range(1, NFB):
                        for hp in range(2):
                            enqueue_block(fb, hp)
                            for g in range(NG):
                                logits_step(fb, hp, g,
                                            av_budget=5 if fb < 3 else 8)
                            if hp == 0 and fb < NFB - 1:
                                q_proj(fb + 1)
                    drain_av(10 ** 9)

    nc.compile()
    return nc


_CACHE = {}


def _get_module(with_biases=False):
    key = ("nc", with_biases)
    if key not in _CACHE:
        _CACHE[key] = build_kernel(with_biases=with_biases)
    return _CACHE[key]


def make_in_maps(query, memory, memory_bias, Wq, bq, Wk, bk, Wv, bv, Wo, bo,
                 mm_np=None, with_biases=False):
    if mm_np is None:
        import ml_dtypes
        mm_np = ml_dtypes.bfloat16
    query = np.asarray(query, np.float32)
    memory = np.asarray(memory, np.float32)
    memory_bias = np.asarray(memory_bias, np.float32)
    Wq = np.asarray(Wq, np.float32)
    bq = np.asarray(bq, np.float32)
    Wk = np.asarray(Wk, np.float32)
    bk = np.asarray(bk, np.float32)
    Wv = np.asarray(Wv, np.float32)
    bv = np.asarray(bv, np.float32)
    Wo = np.asarray(Wo, np.float32)
    s = np.float32(DH ** -0.5)

    qT = [np.ascontiguousarray(query[b].T).astype(mm_np) for b in range(B)]
    mT = [np.ascontiguousarray(memory[b].T).astype(mm_np) for b in range(B)]
    in_maps = []
    for c in range(NCORE):
        b, g = divmod(c, 4)
        J = slice(g * JC, (g + 1) * JC)
        m = {
            "qT": qT[b],
            "mT": mT[b],
            "wq": (np.ascontiguousarray(Wq[:, J]) * s).astype(mm_np),
            "wk": np.ascontiguousarray(Wk[:, J]).astype(mm_np),
            "wv": np.ascontiguousarray(Wv[:, J]).astype(mm_np),
            "wo": np.ascontiguousarray(Wo[J, :]).astype(mm_np),
            "eb": np.ascontiguousarray(
                np.exp(memory_bias[b].astype(np.float64)).reshape(
                    NTC, P).T).astype(np.float32),
        }
        if with_biases:
            m["bq"] = (bq[J] * s).reshape(1, JC).astype(mm_np)
            m["bk"] = bk[J].reshape(1, JC).astype(mm_np)
            m["bv"] = bv[J].reshape(1, JC).astype(mm_np)
        in_maps.append(m)
    return in_maps


def gather_output(results, bo):
    bo = np.asarray(bo, np.float32)
    out = np.empty((B, LQ, D), np.float32)
    for b in range(B):
        acc = results[4 * b]["out"].astype(np.float32)
        for g in range(1, 4):
            acc = acc + results[4 * b + g]["out"]
        out[b] = acc + bo
    return out


def kernel(**inputs):
    wb = any(np.any(np.asarray(inputs[b])) for b in ("bq", "bk", "bv"))
    nc = _get_module(with_biases=wb)
    in_maps = make_in_maps(**inputs, with_biases=wb)
    res = bass_utils.run_bass_kernel_spmd(nc, in_maps,
                                          core_ids=list(range(NCORE)))
    return gather_output(res.results, inputs["bo"])


# revision 35
# speedup vs baseline: 1.1969x; 1.0069x over previous
"""Multi-head attention (B=2, L=2048, D=1024, H=16) on 8 TRN2 NeuronCores.

Sharding: batch (2) x head-group (4 heads each) = 8 shards.
Each core computes q/k/v projections for its 4 heads, attention, and a
partial output projection (its 256 rows of Wo); host sums the 4 partials
per batch and adds bo.

Device dataflow (per core):
  inputs (host-prepped, bf16 except memory_bias):
    qT  [1024, 2048]  = query[b].T          (d on partitions for matmul)
    mT  [1024, 2048]  = memory[b].T
    wq  [1024, 256]   = Wq[:, J] * 0.125    (scale folded)
    wk, wv [1024, 256];  bq*0.125, bk, bv [1, 256]
    wo  [256, 1024]   = Wo[J, :]
    mb  [16, 128]     = memory_bias[b]  (f32)
  phase 1: qT_h [j, f], kT_h [j, t] (head-dim on partitions), v [t, j]
           (natural), biases folded in via K=1 ones matmuls, and
           v scaled by exp(memory_bias[t]) so the softmax bias drops out:
           softmax(s + b) @ v == (exp(s) @ (v * e^b)) / (exp(s) @ e^b)
  phase 2: sT = kT.T@qT (two heads row-tiled to overlap in the PE array)
           -> exp on ScalarE (the phase-2 bottleneck, kept saturated via
           a triple-buffered psum pool) -> AV matmul with an extra
           e^b column producing the softmax denominator row for free ->
           normalize into a [128, f] head-pair tile -> row-paired output
           projection partial [f, j].
"""

import numpy as np

import concourse.bass as bass
import concourse.tile as tile
from concourse import bacc, mybir
from concourse import bass_utils
from concourse.bass import ts, ds

F32 = mybir.dt.float32
F32R = mybir.dt.float32r
BF16 = mybir.dt.bfloat16

B, LQ, LM, D, H = 2, 2048, 2048, 1024, 16
DH = 64
HPC = 4            # heads per core
JC = HPC * DH      # 256 projection cols per core
NCORE = 8
P = 128
FB = 512           # f-block width
NFB = LQ // FB     # 4
NTC = LM // P      # 16 t-chunks
KD = D // P        # 8 contraction chunks for projections
G = 2              # t-chunks per exp group (psum tile [128, G*512])
NG = NTC // G      # 8 groups

VW = DH + 1        # v columns per head incl. denominator column
VS = DH + 2        # v column stride per head (4B alignment in bf16)


def build_kernel(mm_dt=BF16, with_biases=False):
    MM = mm_dt
    nc = bacc.Bacc("TRN2", target_bir_lowering=False, debug=False)

    qTd = nc.dram_tensor("qT", [D, LQ], MM, kind="ExternalInput").ap()
    mTd = nc.dram_tensor("mT", [D, LM], MM, kind="ExternalInput").ap()
    wqd = nc.dram_tensor("wq", [D, JC], MM, kind="ExternalInput").ap()
    wkd = nc.dram_tensor("wk", [D, JC], MM, kind="ExternalInput").ap()
    wvd = nc.dram_tensor("wv", [D, JC], MM, kind="ExternalInput").ap()
    if with_biases:
        bqd = nc.dram_tensor("bq", [1, JC], MM, kind="ExternalInput").ap()
        bkd = nc.dram_tensor("bk", [1, JC], MM, kind="ExternalInput").ap()
        bvd = nc.dram_tensor("bv", [1, JC], MM, kind="ExternalInput").ap()
    wod = nc.dram_tensor("wo", [JC, D], MM, kind="ExternalInput").ap()
    ebd = nc.dram_tensor("eb", [P, NTC], F32, kind="ExternalInput").ap()
    outd = nc.dram_tensor("out", [LQ, D], F32, kind="ExternalOutput").ap()

    with tile.TileContext(nc) as tc:
        with (
            tc.tile_pool(name="persist", bufs=1) as persist,
            tc.tile_pool(name="vpool", bufs=1) as vpool,
            tc.tile_pool(name="consts", bufs=1) as consts,
        ):
            # ---- constants ----
            ones_f = consts.tile([1, FB], F32)
            nc.vector.memset(ones_f[:], 1.0)
            ones_row = consts.tile([1, FB], MM)      # rhs for bias matmuls
            nc.vector.tensor_copy(ones_row[:], ones_f[:])
            ones_col = consts.tile([1, P], MM)       # lhsT for v-bias / R bcast
            nc.vector.tensor_copy(ones_col[:], ones_f[:, 0:P])
            eb_sb = consts.tile([P, NTC], F32)  # exp(memory_bias), col=tc
            nc.gpsimd.dma_start(eb_sb[:], ebd[:])

            # ---- persistent activations ----
            # qT/kT: per head-pair tile [128 (2 heads x 64 dh), L]
            qTp = [persist.tile([P, LQ], MM, name=f"qTp{i}") for i in range(2)]
            kTp = [persist.tile([P, LM], MM, name=f"kTp{i}") for i in range(2)]
            # v: per t-chunk [128 t, 4 heads x (64 v cols + e^b col + pad)]
            v_sb = [vpool.tile([P, HPC * VS], MM, name=f"v{t}")
                    for t in range(NTC)]
            wop = [persist.tile([P, D], MM, name=f"wop{i}")
                   for i in range(2)]

            # ======= fused projections + attention, software-pipelined =======
            # Emission order (per-engine streams are in-order, so emission
            # order is the schedule):
            #   rounds 0..3:  k/v projections for t-window r, q projection
            #                 for f-block 0 (round 0 only), then fb0's logits
            #                 groups {2r, 2r+1} for both head pairs
            #   fb blocks 1..3: q projection, then 16 logits+exp steps, with
            #                 AV matmuls of previously-completed (fb, hp)
            #                 blocks drained from a FIFO at 4 per step
            #   tail:         remaining AV units + last out-projection
            # PSUM: psl 3x[128, G*FB] (logits pairs + proj/rb/out-proj
            # rotations) + ppv 1 (v, phase 1 only) + psx 2 (AV accumulators).
            wq_sb = persist.tile([P, KD * JC], MM, name="wq")
            bq_sb = persist.tile([1, JC], MM, name="bq")
            wk_sb = persist.tile([P, KD * JC], MM, name="wk")
            wv_sb = persist.tile([P, KD * JC], MM, name="wv")
            bk_sb = persist.tile([1, JC], MM, name="bk")
            bv_sb = persist.tile([1, JC], MM, name="bv")

            with (
                tc.tile_pool(name="mrhs", bufs=2) as mrhsp,
                tc.tile_pool(name="expp", bufs=34) as expp,
                tc.tile_pool(name="attnp", bufs=6) as attnp,
                tc.tile_pool(name="rp", bufs=6) as rp,
                tc.tile_pool(name="rbp", bufs=4) as rbp,
                tc.tile_pool(name="osb", bufs=4) as osb,
                tc.tile_pool(name="psl", bufs=3, space="PSUM") as psl,
            ):
                for w_sb, wd in ((wk_sb, wkd), (wv_sb, wvd)):
                    for k in range(KD):
                        eng = nc.sync if k % 2 == 0 else nc.gpsimd
                        eng.dma_start(w_sb[:, ts(k, JC)],
                                      wd[ds(k * P, P), :])
                if with_biases:
                    nc.gpsimd.dma_start(bk_sb[:], bkd[:])
                    nc.gpsimd.dma_start(bv_sb[:], bvd[:])
                def late_weight_loads():
                    # issued after round 0's input chunks so the big wq/wo
                    # transfers don't block the first k-projection loads
                    h_ = KD // 2
                    nc.sync.dma_start(
                        wq_sb[:, 0:h_ * JC].rearrange("p (k j) -> p k j",
                                                      k=h_),
                        wqd[ds(0, h_ * P), :].rearrange("(k p) j -> p k j",
                                                        p=P))
                    nc.gpsimd.dma_start(
                        wq_sb[:, h_ * JC:].rearrange("p (k j) -> p k j",
                                                     k=h_),
                        wqd[ds(h_ * P, h_ * P), :].rearrange(
                            "(k p) j -> p k j", p=P))
                    if with_biases:
                        nc.gpsimd.dma_start(bq_sb[:], bqd[:])
                    for i in range(2):
                        nc.gpsimd.dma_start(wop[i][:], wod[ds(i * P, P), :])

                exps = {}      # (fb, hp, h2, g) -> [128, G*FB] bf16 tile
                apairs = {}    # (fb, hp) -> [128, FB] attn pair tile
                work_q = []    # FIFO: ["av", fb, hp, h2, tc_next, av_ap]
                               #       ["op", fb, fc]
                avail = {}     # (fb, hp) -> highest t-chunk with exp emitted

                def kq_proj(w_sb, b_sb, chunks, dstp, col):
                    for hp in range(2):
                        ps = psl.tile([P, G * FB], F32, name="pp", tag="pls")
                        for k in range(KD):
                            nc.tensor.matmul(
                                ps[:, 0:FB],
                                w_sb[:, ds(k * JC + hp * P, P)], chunks[k],
                                start=(k == 0),
                                stop=(not with_biases and k == KD - 1))
                        if with_biases:
                            nc.tensor.matmul(
                                ps[:, 0:FB], b_sb[:, ds(hp * P, P)],
                                ones_row[:], start=False, stop=True)
                        nc.vector.tensor_copy(dstp[hp][:, col], ps[:, 0:FB])

                def q_proj(fb):
                    qt = mrhsp.tile([P, KD * FB], MM, name="qchunk")
                    for k in range(KD):
                        eng = nc.sync if k % 2 == 0 else nc.gpsimd
                        eng.dma_start(qt[:, ts(k, FB)],
                                      qTd[ds(k * P, P), ts(fb, FB)])
                    kq_proj(wq_sb, bq_sb, [qt[:, ts(k, FB)] for k in range(KD)],
                            qTp, ts(fb, FB))

                def finish_unit(u):
                    _, fb, hp, h2, _, av = u
                    dn = rp.tile([1, FB], F32, name="dn")
                    nc.vector.tensor_copy(dn[:], av[ds(DH, 1), :])
                    rf = rp.tile([1, FB], F32, name="rf")
                    nc.vector.reciprocal_approx_fast(rf[:], dn[:])
                    rrow = rp.tile([1, FB], MM, name="rrow")
                    nc.vector.tensor_copy(rrow[:], rf[:])
                    rb_ps = psl.tile([P, G * FB], F32, name="rbps", tag="pls")
                    nc.tensor.matmul(rb_ps[:, 0:FB], ones_col[:], rrow[:],
                                     start=True, stop=True)
                    rb = rbp.tile([DH, FB], F32, name="rb")
                    nc.vector.tensor_copy(rb[:], rb_ps[0:DH, 0:FB])
                    if (fb, hp) not in apairs:
                        apairs[(fb, hp)] = attnp.tile([P, FB], MM,
                                                      name="apair")
                    nc.vector.tensor_tensor(
                        apairs[(fb, hp)][ds(h2 * DH, DH), :],
                        av[0:DH, :], rb[:], op=mybir.AluOpType.mult)
                    if hp == 1 and h2 == 1:
                        for fc in range(4):
                            work_q.append(["op", fb, fc])

                def out_proj_piece(fb, fc):
                    attn = [apairs[(fb, 0)], apairs[(fb, 1)]]
                    o = osb.tile([P, D], F32, name="osb")
                    for jb in range(2):
                        ops = psl.tile([P, G * FB], F32, name="ops",
                                       tag="pls")
                        for hp in range(2):
                            nc.tensor.matmul(
                                ops[:, 0:FB],
                                attn[hp][:, ds(fc * P, P)],
                                wop[hp][:, ts(jb, FB)],
                                start=(hp == 0), stop=(hp == 1))
                        nc.vector.tensor_copy(o[:, ts(jb, FB)],
                                              ops[:, 0:FB])
                    eng = nc.gpsimd if fc % 2 == 0 else nc.sync
                    eng.dma_start(outd[ds(fb * FB + fc * P, P), :], o[:])
                    if fc == 3:
                        apairs.pop((fb, 0))
                        apairs.pop((fb, 1))

                def drain_av(budget):
                    while budget > 0 and work_q:
                        u = work_q[0]
                        if u[0] == "op":
                            out_proj_piece(u[1], u[2])
                            work_q.pop(0)
                            budget -= 3
                            continue
                        _, fb, hp, h2, tcn, av = u
                        if tcn >= avail.get((fb, hp), 0):
                            break   # strict FIFO; head not yet runnable
                        h = 2 * hp + h2
                        nc.tensor.matmul(
                            av[:], v_sb[tcn][:, ds(h * VS, VW)],
                            exps[(fb, hp, h2, tcn // G)][:, ts(tcn % G, FB)],
                            start=(tcn == 0), stop=(tcn == NTC - 1))
                        u[4] += 1
                        budget -= 1
                        if u[4] == NTC:
                            work_q.pop(0)
                            finish_unit(u)

                def logits_step(fb, hp, g, av_budget=4):
                    pls = [psl.tile([P, G * FB], F32, name="pls")
                           for _ in range(2)]
                    for s in range(G):
                        t = g * G + s
                        for h2 in range(2):
                            nc.tensor.matmul(
                                pls[h2][:, ts(s, FB)],
                                kTp[hp][ds(h2 * DH, DH), ts(t, P)],
                                qTp[hp][ds(h2 * DH, DH), ts(fb, FB)],
                                start=True, stop=True)
                    for h2 in range(2):
                        e = expp.tile([P, G * FB], MM, name="exps")
                        nc.scalar.activation(e[:], pls[h2][:],
                                             mybir.ActivationFunctionType.Exp)
                        exps[(fb, hp, h2, g)] = e
                    avail[(fb, hp)] = (g + 1) * G
                    drain_av(av_budget)

                def enqueue_block(fb, hp):
                    for h2 in range(2):
                        av = psx.tile([P, FB], F32, name="av")[0:VW, :]
                        work_q.append(["av", fb, hp, h2, 0, av])

                # ---- phase 1: k/v rounds + fb0 logits ----
                with tc.tile_pool(name="ppv", bufs=1, space="PSUM") as ppv:
                    for rnd in range(NFB):
                        mt = mrhsp.tile([P, KD * FB], MM, name="mchunk")
                        for k in range(KD):
                            eng = nc.sync if k % 2 == 0 else nc.gpsimd
                            eng.dma_start(mt[:, ts(k, FB)],
                                          mTd[ds(k * P, P), ts(rnd, FB)])
                        chunks = [mt[:, ts(k, FB)] for k in range(KD)]
                        if rnd == 0:
                            late_weight_loads()
                        kq_proj(wk_sb, bk_sb, chunks, kTp, ts(rnd, FB))
                        if rnd == 0:
                            q_proj(0)
                        lsteps = [(g, hp) for g in (2 * rnd, 2 * rnd + 1)
                                  for hp in range(2)]
                        for s in range(4):
                            g, hp = lsteps[s]
                            logits_step(0, hp, g)
                            t = rnd * 4 + s
                            psv = ppv.tile([P, JC], F32)
                            for k in range(KD):
                                nc.tensor.matmul(
                                    psv[:], chunks[k][:, ds(s * P, P)],
                                    wv_sb[:, ts(k, JC)],
                                    start=(k == 0),
                                    stop=(not with_biases and k == KD - 1))
                            if with_biases:
                                nc.tensor.matmul(
                                    psv[:], ones_col[:], bv_sb[:],
                                    start=False, stop=True)
                            dst = v_sb[t].rearrange("p (h c) -> p h c", h=HPC)
                            nc.vector.tensor_scalar_mul(
                                dst[:, :, 0:DH],
                                psv[:].rearrange("p (h c) -> p h c", h=HPC),
                                eb_sb[:, ds(t, 1)])
                            for h in range(HPC):
                                nc.vector.tensor_copy(
                                    dst[:, ds(h, 1), ds(DH, 1)],
                                    eb_sb[:, ds(t, 1)])

                # ---- steady state: fb blocks with AV drained in-stream ----
                with tc.tile_pool(name="psx", bufs=2, space="PSUM") as psx:
                    for hp in range(2):
                        enqueue_block(0, hp)
                    q_proj(1)
                    for fb in range(1, NFB):
                        for hp in range(2):
                            enqueue_block(fb, hp)
                            for g in range(NG):
                                logits_step(fb, hp, g,
                                            av_budget=5 if fb < 3 else 8)
                            if hp == 0 and fb < NFB - 1:
                                q_proj(fb + 1)
                    drain_av(10 ** 9)

    nc.compile()
    return nc


_CACHE = {}


def _get_module(with_biases=False):
    key = ("nc", with_biases)
    if key not in _CACHE:
        _CACHE[key] = build_kernel(with_biases=with_biases)
    return _CACHE[key]


def make_in_maps(query, memory, memory_bias, Wq, bq, Wk, bk, Wv, bv, Wo, bo,
                 mm_np=None, with_biases=False):
    if mm_np is None:
        import ml_dtypes
        mm_np = ml_dtypes.bfloat16
    query = np.asarray(query, np.float32)
    memory = np.asarray(memory, np.float32)
    memory_bias = np.asarray(memory_bias, np.float32)
    Wq = np.asarray(Wq, np.float32)
    bq = np.asarray(bq, np.float32)
    Wk = np.asarray(Wk, np.float32)
    bk = np.asarray(bk, np.float32)
    Wv = np.asarray(Wv, np.float32)
    bv = np.asarray(bv, np.float32)
    Wo = np.asarray(Wo, np.float32)
    s = np.float32(DH ** -0.5)

    qT = [np.ascontiguousarray(query[b].T).astype(mm_np) for b in range(B)]
    mT = [np.ascontiguousarray(memory[b].T).astype(mm_np) for b in range(B)]
    in_maps = []
    for c in range(NCORE):
        b, g = divmod(c, 4)
        J = slice(g * JC, (g + 1) * JC)
        m = {
            "qT": qT[b],
            "mT": mT[b],
            "wq": (np.ascontiguousarray(Wq[:, J]) * s).astype(mm_np),
            "wk": np.ascontiguousarray(Wk[:, J]).astype(mm_np),
            "wv": np.ascontiguousarray(Wv[:, J]).astype(mm_np),
            "wo": np.ascontiguousarray(Wo[J, :]).astype(mm_np),
            "eb": np.ascontiguousarray(
                np.exp(memory_bias[b].astype(np.float64)).reshape(
                    NTC, P).T).astype(np.float32),
        }
        if with_biases:
            m["bq"] = (bq[J] * s).reshape(1, JC).astype(mm_np)
            m["bk"] = bk[J].reshape(1, JC).astype(mm_np)
            m["bv"] = bv[J].reshape(1, JC).astype(mm_np)
        in_maps.append(m)
    return in_maps


def gather_output(results, bo):
    bo = np.asarray(bo, np.float32)
    out = np.empty((B, LQ, D), np.float32)
    for b in range(B):
        acc = results[4 * b]["out"].astype(np.float32)
        for g in range(1, 4):
            acc = acc + results[4 * b + g]["out"]
        out[b] = acc + bo
    return out


def kernel(**inputs):
    wb = any(np.any(np.asarray(inputs[b])) for b in ("bq", "bk", "bv"))
    nc = _get_module(with_biases=wb)
    in_maps = make_in_maps(**inputs, with_biases=wb)
    res = bass_utils.run_bass_kernel_spmd(nc, in_maps,
                                          core_ids=list(range(NCORE)))
    return gather_output(res.results, inputs["bo"])


# revision 36
# speedup vs baseline: 1.2034x; 1.0054x over previous
"""Multi-head attention (B=2, L=2048, D=1024, H=16) on 8 TRN2 NeuronCores.

Sharding: batch (2) x head-group (4 heads each) = 8 shards.
Each core computes q/k/v projections for its 4 heads, attention, and a
partial output projection (its 256 rows of Wo); host sums the 4 partials
per batch and adds bo.

Device dataflow (per core):
  inputs (host-prepped, bf16 except memory_bias):
    qT  [1024, 2048]  = query[b].T          (d on partitions for matmul)
    mT  [1024, 2048]  = memory[b].T
    wq  [1024, 256]   = Wq[:, J] * 0.125    (scale folded)
    wk, wv [1024, 256];  bq*0.125, bk, bv [1, 256]
    wo  [256, 1024]   = Wo[J, :]
    mb  [16, 128]     = memory_bias[b]  (f32)
  phase 1: qT_h [j, f], kT_h [j, t] (head-dim on partitions), v [t, j]
           (natural), biases folded in via K=1 ones matmuls, and
           v scaled by exp(memory_bias[t]) so the softmax bias drops out:
           softmax(s + b) @ v == (exp(s) @ (v * e^b)) / (exp(s) @ e^b)
  phase 2: sT = kT.T@qT (two heads row-tiled to overlap in the PE array)
           -> exp on ScalarE (the phase-2 bottleneck, kept saturated via
           a triple-buffered psum pool) -> AV matmul with an extra
           e^b column producing the softmax denominator row for free ->
           normalize into a [128, f] head-pair tile -> row-paired output
           projection partial [f, j].
"""

import numpy as np

import concourse.bass as bass
import concourse.tile as tile
from concourse import bacc, mybir
from concourse import bass_utils
from concourse.bass import ts, ds

F32 = mybir.dt.float32
F32R = mybir.dt.float32r
BF16 = mybir.dt.bfloat16
FP16 = mybir.dt.float16

B, LQ, LM, D, H = 2, 2048, 2048, 1024, 16
DH = 64
HPC = 4            # heads per core
JC = HPC * DH      # 256 projection cols per core
NCORE = 8
P = 128
FB = 512           # f-block width
NFB = LQ // FB     # 4
NTC = LM // P      # 16 t-chunks
KD = D // P        # 8 contraction chunks for projections
G = 2              # t-chunks per exp group (psum tile [128, G*512])
NG = NTC // G      # 8 groups

VW = DH + 1        # v columns per head incl. denominator column
VS = DH + 2        # v column stride per head (4B alignment in bf16)


def build_kernel(mm_dt=FP16, with_biases=False):
    MM = mm_dt
    nc = bacc.Bacc("TRN2", target_bir_lowering=False, debug=False)

    qTd = nc.dram_tensor("qT", [D, LQ], MM, kind="ExternalInput").ap()
    mTd = nc.dram_tensor("mT", [D, LM], MM, kind="ExternalInput").ap()
    wqd = nc.dram_tensor("wq", [D, JC], MM, kind="ExternalInput").ap()
    wkd = nc.dram_tensor("wk", [D, JC], MM, kind="ExternalInput").ap()
    wvd = nc.dram_tensor("wv", [D, JC], MM, kind="ExternalInput").ap()
    if with_biases:
        bqd = nc.dram_tensor("bq", [1, JC], MM, kind="ExternalInput").ap()
        bkd = nc.dram_tensor("bk", [1, JC], MM, kind="ExternalInput").ap()
        bvd = nc.dram_tensor("bv", [1, JC], MM, kind="ExternalInput").ap()
    wod = nc.dram_tensor("wo", [JC, D], MM, kind="ExternalInput").ap()
    ebd = nc.dram_tensor("eb", [P, NTC], F32, kind="ExternalInput").ap()
    outd = nc.dram_tensor("out", [LQ, D], F32, kind="ExternalOutput").ap()

    with tile.TileContext(nc) as tc:
        with (
            tc.tile_pool(name="persist", bufs=1) as persist,
            tc.tile_pool(name="vpool", bufs=1) as vpool,
            tc.tile_pool(name="consts", bufs=1) as consts,
        ):
            # ---- constants ----
            ones_f = consts.tile([1, FB], F32)
            nc.vector.memset(ones_f[:], 1.0)
            ones_row = consts.tile([1, FB], MM)      # rhs for bias matmuls
            nc.vector.tensor_copy(ones_row[:], ones_f[:])
            ones_col = consts.tile([1, P], MM)       # lhsT for v-bias / R bcast
            nc.vector.tensor_copy(ones_col[:], ones_f[:, 0:P])
            eb_sb = consts.tile([P, NTC], F32)  # exp(memory_bias), col=tc
            nc.gpsimd.dma_start(eb_sb[:], ebd[:])

            # ---- persistent activations ----
            # qT/kT: per head-pair tile [128 (2 heads x 64 dh), L]
            qTp = [persist.tile([P, LQ], MM, name=f"qTp{i}") for i in range(2)]
            kTp = [persist.tile([P, LM], MM, name=f"kTp{i}") for i in range(2)]
            # v: per t-chunk [128 t, 4 heads x (64 v cols + e^b col + pad)]
            v_sb = [vpool.tile([P, HPC * VS], MM, name=f"v{t}")
                    for t in range(NTC)]
            wop = [persist.tile([P, D], MM, name=f"wop{i}")
                   for i in range(2)]

            # ======= fused projections + attention, software-pipelined =======
            # Emission order (per-engine streams are in-order, so emission
            # order is the schedule):
            #   rounds 0..3:  k/v projections for t-window r, q projection
            #                 for f-block 0 (round 0 only), then fb0's logits
            #                 groups {2r, 2r+1} for both head pairs
            #   fb blocks 1..3: q projection, then 16 logits+exp steps, with
            #                 AV matmuls of previously-completed (fb, hp)
            #                 blocks drained from a FIFO at 4 per step
            #   tail:         remaining AV units + last out-projection
            # PSUM: psl 3x[128, G*FB] (logits pairs + proj/rb/out-proj
            # rotations) + ppv 1 (v, phase 1 only) + psx 2 (AV accumulators).
            wq_sb = persist.tile([P, KD * JC], MM, name="wq")
            bq_sb = persist.tile([1, JC], MM, name="bq")
            wk_sb = persist.tile([P, KD * JC], MM, name="wk")
            wv_sb = persist.tile([P, KD * JC], MM, name="wv")
            bk_sb = persist.tile([1, JC], MM, name="bk")
            bv_sb = persist.tile([1, JC], MM, name="bv")

            with (
                tc.tile_pool(name="mrhs", bufs=2) as mrhsp,
                tc.tile_pool(name="expp", bufs=34) as expp,
                tc.tile_pool(name="attnp", bufs=6) as attnp,
                tc.tile_pool(name="rp", bufs=6) as rp,
                tc.tile_pool(name="rbp", bufs=4) as rbp,
                tc.tile_pool(name="osb", bufs=4) as osb,
                tc.tile_pool(name="psl", bufs=3, space="PSUM") as psl,
            ):
                for w_sb, wd in ((wk_sb, wkd), (wv_sb, wvd)):
                    for k in range(KD):
                        eng = nc.sync if k % 2 == 0 else nc.gpsimd
                        eng.dma_start(w_sb[:, ts(k, JC)],
                                      wd[ds(k * P, P), :])
                if with_biases:
                    nc.gpsimd.dma_start(bk_sb[:], bkd[:])
                    nc.gpsimd.dma_start(bv_sb[:], bvd[:])
                def late_weight_loads():
                    # issued after round 0's input chunks so the big wq/wo
                    # transfers don't block the first k-projection loads
                    h_ = KD // 2
                    nc.sync.dma_start(
                        wq_sb[:, 0:h_ * JC].rearrange("p (k j) -> p k j",
                                                      k=h_),
                        wqd[ds(0, h_ * P), :].rearrange("(k p) j -> p k j",
                                                        p=P))
                    nc.gpsimd.dma_start(
                        wq_sb[:, h_ * JC:].rearrange("p (k j) -> p k j",
                                                     k=h_),
                        wqd[ds(h_ * P, h_ * P), :].rearrange(
                            "(k p) j -> p k j", p=P))
                    if with_biases:
                        nc.gpsimd.dma_start(bq_sb[:], bqd[:])
                    for i in range(2):
                        nc.gpsimd.dma_start(wop[i][:], wod[ds(i * P, P), :])

                exps = {}      # (fb, hp, h2, g) -> [128, G*FB] bf16 tile
                apairs = {}    # (fb, hp) -> [128, FB] attn pair tile
                work_q = []    # FIFO: ["av", fb, hp, h2, tc_next, av_ap]
                               #       ["op", fb, fc]
                avail = {}     # (fb, hp) -> highest t-chunk with exp emitted

                def kq_proj(w_sb, b_sb, chunks, dstp, col):
                    for hp in range(2):
                        ps = psl.tile([P, G * FB], F32, name="pp", tag="pls")
                        for k in range(KD):
                            nc.tensor.matmul(
                                ps[:, 0:FB],
                                w_sb[:, ds(k * JC + hp * P, P)], chunks[k],
                                start=(k == 0),
                                stop=(not with_biases and k == KD - 1))
                        if with_biases:
                            nc.tensor.matmul(
                                ps[:, 0:FB], b_sb[:, ds(hp * P, P)],
                                ones_row[:], start=False, stop=True)
                        nc.vector.tensor_copy(dstp[hp][:, col], ps[:, 0:FB])

                def q_proj(fb):
                    qt = mrhsp.tile([P, KD * FB], MM, name="qchunk")
                    for k in range(KD):
                        eng = nc.sync if k % 2 == 0 else nc.gpsimd
                        eng.dma_start(qt[:, ts(k, FB)],
                                      qTd[ds(k * P, P), ts(fb, FB)])
                    kq_proj(wq_sb, bq_sb, [qt[:, ts(k, FB)] for k in range(KD)],
                            qTp, ts(fb, FB))

                def finish_unit(u):
                    _, fb, hp, h2, _, av = u
                    dn = rp.tile([1, FB], F32, name="dn")
                    nc.vector.tensor_copy(dn[:], av[ds(DH, 1), :])
                    rf = rp.tile([1, FB], F32, name="rf")
                    nc.vector.reciprocal_approx_fast(rf[:], dn[:])
                    rrow = rp.tile([1, FB], MM, name="rrow")
                    nc.vector.tensor_copy(rrow[:], rf[:])
                    rb_ps = psl.tile([P, G * FB], F32, name="rbps", tag="pls")
                    nc.tensor.matmul(rb_ps[:, 0:FB], ones_col[:], rrow[:],
                                     start=True, stop=True)
                    rb = rbp.tile([DH, FB], F32, name="rb")
                    nc.vector.tensor_copy(rb[:], rb_ps[0:DH, 0:FB])
                    if (fb, hp) not in apairs:
                        apairs[(fb, hp)] = attnp.tile([P, FB], MM,
                                                      name="apair")
                    nc.vector.tensor_tensor(
                        apairs[(fb, hp)][ds(h2 * DH, DH), :],
                        av[0:DH, :], rb[:], op=mybir.AluOpType.mult)
                    if hp == 1 and h2 == 1:
                        for fc in range(4):
                            work_q.append(["op", fb, fc])

                def out_proj_piece(fb, fc):
                    attn = [apairs[(fb, 0)], apairs[(fb, 1)]]
                    o = osb.tile([P, D], F32, name="osb")
                    for jb in range(2):
                        ops = psl.tile([P, G * FB], F32, name="ops",
                                       tag="pls")
                        for hp in range(2):
                            nc.tensor.matmul(
                                ops[:, 0:FB],
                                attn[hp][:, ds(fc * P, P)],
                                wop[hp][:, ts(jb, FB)],
                                start=(hp == 0), stop=(hp == 1))
                        nc.vector.tensor_copy(o[:, ts(jb, FB)],
                                              ops[:, 0:FB])
                    eng = nc.gpsimd if fc % 2 == 0 else nc.sync
                    eng.dma_start(outd[ds(fb * FB + fc * P, P), :], o[:])
                    if fc == 3:
                        apairs.pop((fb, 0))
                        apairs.pop((fb, 1))

                def drain_av(budget):
                    while budget > 0 and work_q:
                        u = work_q[0]
                        if u[0] == "op":
                            out_proj_piece(u[1], u[2])
                            work_q.pop(0)
                            budget -= 3
                            continue
                        _, fb, hp, h2, tcn, av = u
                        if tcn >= avail.get((fb, hp), 0):
                            break   # strict FIFO; head not yet runnable
                        h = 2 * hp + h2
                        nc.tensor.matmul(
                            av[:], v_sb[tcn][:, ds(h * VS, VW)],
                            exps[(fb, hp, h2, tcn // G)][:, ts(tcn % G, FB)],
                            start=(tcn == 0), stop=(tcn == NTC - 1))
                        u[4] += 1
                        budget -= 1
                        if u[4] == NTC:
                            work_q.pop(0)
                            finish_unit(u)

                def logits_step(fb, hp, g, av_budget=4):
                    pls = [psl.tile([P, G * FB], F32, name="pls")
                           for _ in range(2)]
                    for s in range(G):
                        t = g * G + s
                        for h2 in range(2):
                            nc.tensor.matmul(
                                pls[h2][:, ts(s, FB)],
                                kTp[hp][ds(h2 * DH, DH), ts(t, P)],
                                qTp[hp][ds(h2 * DH, DH), ts(fb, FB)],
                                start=True, stop=True)
                    for h2 in range(2):
                        e = expp.tile([P, G * FB], MM, name="exps")
                        nc.scalar.activation(e[:], pls[h2][:],
                                             mybir.ActivationFunctionType.Exp)
                        exps[(fb, hp, h2, g)] = e
                    avail[(fb, hp)] = (g + 1) * G
                    drain_av(av_budget)

                def enqueue_block(fb, hp):
                    for h2 in range(2):
                        av = psx.tile([P, FB], F32, name="av")[0:VW, :]
                        work_q.append(["av", fb, hp, h2, 0, av])

                # ---- phase 1: k/v rounds + fb0 logits ----
                with tc.tile_pool(name="ppv", bufs=1, space="PSUM") as ppv:
                    for rnd in range(NFB):
                        mt = mrhsp.tile([P, KD * FB], MM, name="mchunk")
                        for k in range(KD):
                            eng = nc.sync if k % 2 == 0 else nc.gpsimd
                            eng.dma_start(mt[:, ts(k, FB)],
                                          mTd[ds(k * P, P), ts(rnd, FB)])
                        chunks = [mt[:, ts(k, FB)] for k in range(KD)]
                        if rnd == 0:
                            late_weight_loads()
                        kq_proj(wk_sb, bk_sb, chunks, kTp, ts(rnd, FB))
                        if rnd == 0:
                            q_proj(0)
                        lsteps = [(g, hp) for g in (2 * rnd, 2 * rnd + 1)
                                  for hp in range(2)]
                        for s in range(4):
                            g, hp = lsteps[s]
                            logits_step(0, hp, g)
                            t = rnd * 4 + s
                            psv = ppv.tile([P, JC], F32)
                            for k in range(KD):
                                nc.tensor.matmul(
                                    psv[:], chunks[k][:, ds(s * P, P)],
                                    wv_sb[:, ts(k, JC)],
                                    start=(k == 0),
                                    stop=(not with_biases and k == KD - 1))
                            if with_biases:
                                nc.tensor.matmul(
                                    psv[:], ones_col[:], bv_sb[:],
                                    start=False, stop=True)
                            dst = v_sb[t].rearrange("p (h c) -> p h c", h=HPC)
                            nc.vector.tensor_scalar_mul(
                                dst[:, :, 0:DH],
                                psv[:].rearrange("p (h c) -> p h c", h=HPC),
                                eb_sb[:, ds(t, 1)])
                            for h in range(HPC):
                                nc.vector.tensor_copy(
                                    dst[:, ds(h, 1), ds(DH, 1)],
                                    eb_sb[:, ds(t, 1)])

                # ---- steady state: fb blocks with AV drained in-stream ----
                with tc.tile_pool(name="psx", bufs=2, space="PSUM") as psx:
                    for hp in range(2):
                        enqueue_block(0, hp)
                    q_proj(1)
                    for fb in range(1, NFB):
                        for hp in range(2):
                            enqueue_block(fb, hp)
                            for g in range(NG):
                                logits_step(fb, hp, g,
                                            av_budget=5 if fb < 3 else 8)
                            if hp == 0 and fb < NFB - 1:
                                q_proj(fb + 1)
                    drain_av(10 ** 9)

    nc.compile()
    return nc


_CACHE = {}


def _get_module(with_biases=False):
    key = ("nc", with_biases)
    if key not in _CACHE:
        _CACHE[key] = build_kernel(with_biases=with_biases)
    return _CACHE[key]


def make_in_maps(query, memory, memory_bias, Wq, bq, Wk, bk, Wv, bv, Wo, bo,
                 mm_np=None, with_biases=False):
    if mm_np is None:
        mm_np = np.float16
    query = np.asarray(query, np.float32)
    memory = np.asarray(memory, np.float32)
    memory_bias = np.asarray(memory_bias, np.float32)
    Wq = np.asarray(Wq, np.float32)
    bq = np.asarray(bq, np.float32)
    Wk = np.asarray(Wk, np.float32)
    bk = np.asarray(bk, np.float32)
    Wv = np.asarray(Wv, np.float32)
    bv = np.asarray(bv, np.float32)
    Wo = np.asarray(Wo, np.float32)
    s = np.float32(DH ** -0.5)

    qT = [np.ascontiguousarray(query[b].T).astype(mm_np) for b in range(B)]
    mT = [np.ascontiguousarray(memory[b].T).astype(mm_np) for b in range(B)]
    in_maps = []
    for c in range(NCORE):
        b, g = divmod(c, 4)
        J = slice(g * JC, (g + 1) * JC)
        m = {
            "qT": qT[b],
            "mT": mT[b],
            "wq": (np.ascontiguousarray(Wq[:, J]) * s).astype(mm_np),
            "wk": np.ascontiguousarray(Wk[:, J]).astype(mm_np),
            "wv": np.ascontiguousarray(Wv[:, J]).astype(mm_np),
            "wo": np.ascontiguousarray(Wo[J, :]).astype(mm_np),
            "eb": np.ascontiguousarray(
                np.exp(memory_bias[b].astype(np.float64)).reshape(
                    NTC, P).T).astype(np.float32),
        }
        if with_biases:
            m["bq"] = (bq[J] * s).reshape(1, JC).astype(mm_np)
            m["bk"] = bk[J].reshape(1, JC).astype(mm_np)
            m["bv"] = bv[J].reshape(1, JC).astype(mm_np)
        in_maps.append(m)
    return in_maps


def gather_output(results, bo):
    bo = np.asarray(bo, np.float32)
    out = np.empty((B, LQ, D), np.float32)
    for b in range(B):
        acc = results[4 * b]["out"].astype(np.float32)
        for g in range(1, 4):
            acc = acc + results[4 * b + g]["out"]
        out[b] = acc + bo
    return out


def kernel(**inputs):
    wb = any(np.any(np.asarray(inputs[b])) for b in ("bq", "bk", "bv"))
    nc = _get_module(with_biases=wb)
    in_maps = make_in_maps(**inputs, with_biases=wb)
    res = bass_utils.run_bass_kernel_spmd(nc, in_maps,
                                          core_ids=list(range(NCORE)))
    return gather_output(res.results, inputs["bo"])
